# revision 9
# baseline (speedup 1.0000x reference)
"""Dense optical flow kernel for Trainium2, 8-core SPMD.

Pipeline (per core = one (sample, x-half) pair):
  frames (x-polyphase layout q=x%4) -> gray/sobel features in-place
  -> l2-normalize f2 -> 11-slot row-replicated f2px tensor (DMA)
  -> 15x15 windowed correlation in fp32 with ALL operands unit-stride
  (x-polyphase makes every dx shift a contiguous 68-run) -> tree maxes
  -> batched first-argmax -> displacement grid -> separable gaussian
  smoothing (phase H-pass on DVE, banded-matmul V-pass on PE)
  -> direction normalize -> full-res flow.

The x dimension is stored phase-major ([q=x%4][u=x//4]) so the stride-4
anchor/window gathers of the correlation become stride-1 runs; window
shifts dx group by phase q into overlapping-window access patterns
(outer dim stride 1 over an inner stride-1 run) built by AP surgery.
"""

import numpy as np

import concourse.bacc as bacc
import concourse.tile as tile
from concourse import mybir
from concourse.bass_utils import run_bass_kernel_spmd

F32 = mybir.dt.float32
Alu = mybir.AluOpType
Act = mybir.ActivationFunctionType
AX = mybir.AxisListType

H = 512
W = 512
B = 4
XL = 288          # per-core padded column span
U72 = 72          # columns per phase
GXL = 68          # local anchor columns (64 + 2 halo each side)
NEG = np.float32(-1.0e30)
POS = np.float32(3.0e38)


# ----------------------------------------------------------------------------
# constants (host side)
# ----------------------------------------------------------------------------

def _gaussian_sep():
    ax = np.arange(15) - 7
    g = np.exp(-(ax.astype(np.float64) ** 2) / (2.0 * 2.5 ** 2))
    return (g / g.sum())


def _phase_weights():
    g = _gaussian_sep()
    Wp = np.zeros((4, 5), np.float64)
    for p in range(4):
        for t in range(15):
            Wp[p, (p + t - 7) // 4 + 2] += g[t]
    return Wp.astype(np.float32)


def _band_matrices():
    # bands[t][v, y]: out_row(128t+y) = sum_v band[v, y] * hp[v]
    Wp = _phase_weights()
    bands = np.zeros((4, 128, 128), np.float32)
    for t in range(4):
        for y in range(128):
            yg = 128 * t + y
            v0, q = yg // 4, yg % 4
            for d in range(5):
                v = v0 + d - 2
                if 0 <= v < 128:
                    bands[t, v, y] = Wp[q, d]
    return bands


def _phase_major(a):
    # a: (C, 128, 4, XL) -> (128, ry, C, q, u)
    C = a.shape[0]
    return np.ascontiguousarray(
        a.reshape(C, 128, 4, U72, 4).transpose(1, 2, 0, 4, 3))


def _host_inputs(frame1, frame2):
    """Build the 8 per-core input maps."""
    bands = _band_matrices()
    # weight tables for first-argmax (shared across cores)
    wt = np.zeros((2, 16, 1), np.float32)
    for p in range(1, 16):          # ch0: colmax position p = 4q+u0
        q, u0 = p // 4, p % 4
        wt[0, p, 0] = np.float32(16 - 4 * u0 - q)
    for s in range(1, 16):          # ch1: rowmax slot s (i = s-1)
        wt[1, s, 0] = np.float32(16 - s)
    wtab = np.tile(wt, (1, 1, GXL)).reshape(1, 2 * 16 * GXL)
    in_maps = []
    for b in range(B):
        for w in range(2):
            xbase = 256 * w - 16
            sl1 = np.zeros((3, H, XL), np.float32)
            sl2 = np.zeros((3, H, XL), np.float32)
            lo, hi = max(0, xbase), min(W, xbase + XL)
            sl1[:, :, lo - xbase:hi - xbase] = frame1[b][:, :, lo:hi]
            sl2[:, :, lo - xbase:hi - xbase] = frame2[b][:, :, lo:hi]
            il1 = _phase_major(sl1.reshape(3, 128, 4, XL))
            il2 = _phase_major(sl2.reshape(3, 128, 4, XL))
            il1 = np.ascontiguousarray(il1[:, (0, 1, 3)])  # ry 2 unused
            # column-validity mask (phase-major [q, u])
            xcols = xbase + np.arange(XL)
            valid = (xcols >= 0) & (xcols < W)
            xm = np.where(valid, POS, NEG).astype(np.float32)
            xm = np.ascontiguousarray(
                xm.reshape(U72, 4).T).reshape(1, XL)
            # anchor-validity mask
            gxg = 64 * w - 2 + np.arange(GXL)
            gm = ((gxg >= 0) & (gxg < 128)).astype(np.float32)[None, :]
            consts = np.concatenate(
                [np.tile(xm, (128, 1)), np.tile(gm, (128, 1)),
                 bands.transpose(1, 0, 2).reshape(128, 512),
                 np.tile(wtab, (128, 1))], axis=1)
            in_maps.append({"f1s": il1, "f2s": il2,
                            "consts": consts.astype(np.float32)})
    return in_maps


# ----------------------------------------------------------------------------
# device program
# ----------------------------------------------------------------------------

def _win(ap5, s_lo, s_hi, c, q, ubase, nu):
    """Overlapping-window AP: [128, nu, s_hi-s_lo, 68] over a
    [128, S, 3, 4, 72] tensor; dims (u0, slot, gx) strides (1, slotstride, 1)
    reading u = ubase + u0 + gx of phase q, channel c."""
    a = ap5[:, s_lo:s_hi, c, q, ubase:].copy()
    d = a.ap
    d.insert(1, (1, nu))
    d[3] = (1, GXL)
    return a


def _emit_features_ph(nc, raw, sd, R):
    """In-place gray + sobel-H into raw/sd, phase-major layout.

    raw: [128, R, 3, 4, 72] (c0 becomes gray, c1 fx, c2 fy later)
    sd:  [128, R, 2, 4, 72] (idx0 = s, idx1 = d)
    """
    # gray scaled by 1/0.299: uniform feature scale cancels in the
    # per-pixel l2-normalize (sobel is linear), so argmax is unchanged
    g = raw[:][:, :, 0, :, :]                    # [128, R, 4, 72]
    nc.vector.scalar_tensor_tensor(g, raw[:][:, :, 1, :, :],
                                   float(np.float32(0.587 / 0.299)), g,
                                   Alu.mult, Alu.add)
    nc.vector.scalar_tensor_tensor(g, raw[:][:, :, 2, :, :],
                                   float(np.float32(0.114 / 0.299)), g,
                                   Alu.mult, Alu.add)
    s = sd[:][:, :, 0, :, :]
    d = sd[:][:, :, 1, :, :]
    # horizontal sobel, phase-decomposed (x = 4u+q)
    nc.vector.tensor_tensor(d[:, :, 1:3, :], g[:, :, 2:4, :],
                            g[:, :, 0:2, :], Alu.subtract)
    nc.vector.tensor_tensor(d[:, :, 0, 1:U72], g[:, :, 1, 1:U72],
                            g[:, :, 3, 0:U72 - 1], Alu.subtract)
    nc.vector.tensor_tensor(d[:, :, 3, 0:U72 - 1], g[:, :, 0, 1:U72],
                            g[:, :, 2, 0:U72 - 1], Alu.subtract)
    nc.vector.scalar_tensor_tensor(s[:, :, 1:3, :], g[:, :, 1:3, :], 2.0,
                                   g[:, :, 0:2, :], Alu.mult, Alu.add)
    nc.vector.tensor_tensor(s[:, :, 1:3, :], s[:, :, 1:3, :],
                            g[:, :, 2:4, :], Alu.add)
    nc.vector.scalar_tensor_tensor(s[:, :, 0, 1:U72], g[:, :, 0, 1:U72],
                                   2.0, g[:, :, 3, 0:U72 - 1],
                                   Alu.mult, Alu.add)
    nc.vector.tensor_tensor(s[:, :, 0, 1:U72], s[:, :, 0, 1:U72],
                            g[:, :, 1, 1:U72], Alu.add)
    nc.vector.scalar_tensor_tensor(s[:, :, 3, 0:U72 - 1],
                                   g[:, :, 3, 0:U72 - 1], 2.0,
                                   g[:, :, 2, 0:U72 - 1], Alu.mult, Alu.add)
    nc.vector.tensor_tensor(s[:, :, 3, 0:U72 - 1], s[:, :, 3, 0:U72 - 1],
                            g[:, :, 0, 1:U72], Alu.add)
    # boundary columns x=0 (q0,u0) and x=287 (q3,u71): zero s and d
    nc.vector.memset(sd[:][:, :, :, 0, 0:1].squeeze(-1), 0.0)
    nc.vector.memset(sd[:][:, :, :, 3, U72 - 1:U72].squeeze(-1), 0.0)


def build_program():
    nc = bacc.Bacc("TRN2", target_bir_lowering=False, debug=False)

    f1s_d = nc.dram_tensor("f1s", [128, 3, 3, 4, U72], F32,
                           kind="ExternalInput")
    f2s_d = nc.dram_tensor("f2s", [128, 4, 3, 4, U72], F32,
                           kind="ExternalInput")
    NCONST = XL + GXL + 512 + 2 * 16 * GXL
    consts_d = nc.dram_tensor("consts", [128, NCONST], F32,
                              kind="ExternalInput")
    out_d = nc.dram_tensor("out", [128, 4, 2, 256], F32,
                           kind="ExternalOutput")

    with tile.TileContext(nc) as tc:
        with tc.tile_pool(name="main", bufs=1) as pool, \
             tc.tile_pool(name="psum", bufs=4, space="PSUM") as psum_pool:

            raw2 = pool.tile([128, 4, 3, 4, U72], F32)   # becomes feat2
            raw1 = pool.tile([128, 3, 3, 4, U72], F32)   # becomes feat1
            sd2 = pool.tile([128, 4, 2, 4, U72], F32)
            sd1 = pool.tile([128, 3, 2, 4, U72], F32)
            sdm1 = pool.tile([128, 2, 4, U72], F32)
            sdp1 = pool.tile([128, 2, 4, U72], F32)
            sdm1f1 = pool.tile([128, 2, 4, U72], F32)
            consts = pool.tile([128, NCONST], F32)
            q_t = pool.tile([128, 4, 4, U72], F32)
            r0_t = pool.tile([128, 4, 4, U72], F32)
            a_t = pool.tile([128, 4, 4, U72], F32)
            f2px = pool.tile([128, 16, 3, 4, U72], F32)
            # correlation scratch
            corrE = pool.tile([128, 4, 4, GXL], F32)
            corrL = pool.tile([128, 4, 16, GXL], F32, tag="sd2")
            prod = pool.tile([128, 4, 7, GXL], F32, tag="sdp1")
            tA = pool.tile([128, 4, 3, GXL], F32)
            tB = pool.tile([128, 4, GXL], F32)
            mx = pool.tile([128, 2, 16, GXL], F32)
            tr = pool.tile([128, 2, 8, GXL], F32, tag="a_t")
            wsum = pool.tile([128, 2, 16, GXL], F32, tag="sd1")
            m2 = pool.tile([128, 2, GXL], F32)
            fm2 = pool.tile([128, 2, GXL], F32)
            grid = pool.tile([128, 2, GXL], F32)
            hp = pool.tile([128, 2, 256], F32)
            hsc = pool.tile([128, 2, 64], F32)
            tscr = pool.tile([128, 64], F32)
            bands2 = pool.tile([128, 4, 128], F32)
            smsb = [pool.tile([128, 2, 256], F32, name=f"smsb{t}", tag=tg)
                    for t, tg in enumerate(("q_t", "r0_t", "sdm1", "sdm1f1"))]
            nq1s = [pool.tile([128, 256], F32, name=f"nq1_{t}")
                    for t in range(4)]
            nq2s = [pool.tile([128, 256], F32, name=f"nq2_{t}")
                    for t in range(4)]
            nrs = [pool.tile([128, 256], F32, name=f"nr_{t}")
                   for t in range(4)]
            nms = [pool.tile([128, 256], F32, name=f"nm_{t}")
                   for t in range(4)]
            outsb = pool.tile([128, 4, 2, 256], F32, tag="raw1")

            _touch_n = [0]

            def touch(ap):
                # one-wait funnel: absorb a DMA-queue semaphore into the
                # DVE engine clock so consumers carry fewer sync waits
                k = _touch_n[0] = _touch_n[0] + 1
                nc.vector.tensor_copy(tscr[:][32:33, k % 64:k % 64 + 1], ap)

            # ---------------- input DMAs ----------------
            for p0 in range(0, 128, 32):
                nc.sync.dma_start(raw2[:][p0:p0 + 32], f2s_d.ap()[p0:p0 + 32])
            for p0 in range(0, 128, 32):
                nc.sync.dma_start(raw1[:][p0:p0 + 32], f1s_d.ap()[p0:p0 + 32])
            nc.sync.dma_start(consts[:], consts_d.ap())
            touch(consts[:][32:33, 0:1])
            xmask = consts[:][:, 0:XL].rearrange("p (q u) -> p q u", q=4)
            gmask = consts[:][:, XL:XL + GXL]
            bands = consts[:][:, XL + GXL:XL + GXL + 512].rearrange(
                "p (t y) -> p t y", t=4)
            wtab = consts[:][:, XL + GXL + 512:].rearrange(
                "p (c s g) -> p c s g", c=2, s=16)

            # ---------------- frame2 features + normalize ----------------
            _emit_features_ph(nc, raw2, sd2, 4)
            s2v = sd2[:][:, :, 0, :, :]
            d2v = sd2[:][:, :, 1, :, :]
            # cross-partition row shifts for the vertical sobel pass
            nc.vector.memset(sdm1[:][0:1], 0.0)
            nc.gpsimd.dma_start(sdm1[:][1:64], sd2[:][0:63, 3, :, :, :])
            nc.gpsimd.dma_start(sdm1[:][64:128], sd2[:][63:127, 3, :, :, :])
            nc.vector.memset(sdp1[:][96:128], 0.0)
            nc.gpsimd.dma_start(sdp1[:][0:64], sd2[:][1:65, 0, :, :, :])
            nc.gpsimd.dma_start(sdp1[:][64:127], sd2[:][65:128, 0, :, :, :])
            fxp2 = raw2[:][:, :, 1, :, :]
            fyp2 = raw2[:][:, :, 2, :, :]

            def vconv(fxp, fyp, dv, sv, ry, dm1, dp1, sm1, sp1):
                nc.vector.scalar_tensor_tensor(fxp[:, ry], dv[:, ry], 2.0,
                                               dm1, Alu.mult, Alu.add)
                nc.vector.tensor_tensor(fxp[:, ry], fxp[:, ry], dp1, Alu.add)
                nc.vector.tensor_tensor(fyp[:, ry], sp1, sm1, Alu.subtract)

            # ry=1,2 batched (neighbors live inside sd2)
            nc.vector.scalar_tensor_tensor(fxp2[:, 1:3], d2v[:, 1:3], 2.0,
                                           d2v[:, 0:2], Alu.mult, Alu.add)
            nc.vector.tensor_tensor(fxp2[:, 1:3], fxp2[:, 1:3], d2v[:, 2:4],
                                    Alu.add)
            nc.vector.tensor_tensor(fyp2[:, 1:3], s2v[:, 2:4], s2v[:, 0:2],
                                    Alu.subtract)
            vconv(fxp2, fyp2, d2v, s2v, 3, d2v[:, 2], sdp1[:][:, 1],
                  s2v[:, 2], sdp1[:][:, 0])
            vconv(fxp2, fyp2, d2v, s2v, 0, sdm1[:][:, 1], d2v[:, 1],
                  sdm1[:][:, 0], s2v[:, 1])

            # normalize: q = g^2+fx^2+fy^2 (squares on ACT), rsqrt + Newton
            nc.scalar.activation(q_t[:], raw2[:][:, :, 0, :, :], Act.Square)
            nc.scalar.activation(r0_t[:], raw2[:][:, :, 1, :, :], Act.Square)
            nc.scalar.activation(a_t[:], raw2[:][:, :, 2, :, :], Act.Square)
            nc.vector.tensor_tensor(q_t[:], q_t[:], r0_t[:], Alu.add)
            # q = (fy^2 max 1e-24) + (g^2 + fx^2): same zero-pixel guard as
            # max(q, 1e-24) since all terms are >= 0
            nc.vector.scalar_tensor_tensor(q_t[:], a_t[:], 1e-24, q_t[:],
                                           Alu.max, Alu.add)
            nc.scalar.activation(r0_t[:], q_t[:], Act.Abs_reciprocal_sqrt)
            nc.vector.tensor_tensor(a_t[:], r0_t[:], r0_t[:], Alu.mult)
            nc.vector.tensor_tensor(a_t[:], a_t[:], q_t[:], Alu.mult)
            nc.vector.tensor_scalar(a_t[:], a_t[:], -0.5, 1.5, Alu.mult,
                                    Alu.add)
            nc.vector.tensor_tensor(r0_t[:], r0_t[:], a_t[:], Alu.mult)
            for c in range(3):
                nc.vector.tensor_tensor(raw2[:][:, :, c, :, :],
                                        raw2[:][:, :, c, :, :],
                                        r0_t[:], Alu.mult)
            # column-validity mask on the gray plane
            nc.vector.tensor_tensor(
                raw2[:][:, :, 0, :, :], raw2[:][:, :, 0, :, :],
                xmask.unsqueeze(1).broadcast_to([128, 4, 4, U72]), Alu.min)

            # ---------------- f2px replication ----------------
            # out-of-image rows: gray=NEG, fx/fy=0 (overwritten where valid)
            # corner masks on the Pool engine (DVE stays on the main chain)
            f2flat = f2px[:].rearrange("p s c q u -> p s (c q u)")
            for (pa, pb, sa, sb) in ((0, 2, 1, 4), (0, 1, 4, 8),
                                     (96, 128, 12, 16)):
                nc.gpsimd.memset(f2flat[pa:pb, sa:sb, 0:XL], float(NEG))
                nc.gpsimd.memset(f2flat[pa:pb, sa:sb, XL:3 * XL], 0.0)
            # slot s holds rows 4(v+ov)+ry, s = 4*ovi+ovr, ov = ovi-2;
            # ovi=0 pieces ride the SWDGE queues to widen DMA bandwidth
            for (ovi, sa, sb, ra) in ((0, 1, 4, 1), (1, 4, 8, 0),
                                      (3, 12, 16, 0)):
                ov = ovi - 2
                p0, p1 = max(0, -ov), min(128, 128 - ov)
                eng = nc.gpsimd if ovi == 0 else nc.sync
                for q0 in range(0, 128, 32):
                    a, b = max(p0, q0), min(p1, q0 + 32)
                    if a < b:
                        eng.dma_start(
                            f2px[:][a:b, sa:sb, :, :, :],
                            raw2[:][a + ov:b + ov, ra:4, :, :, :])

            # ---------------- frame1 features (anchor rows only) ----------
            _emit_features_ph(nc, raw1, sd1, 3)
            s1v = sd1[:][:, :, 0, :, :]
            d1v = sd1[:][:, :, 1, :, :]
            nc.vector.memset(sdm1f1[:][0:1], 0.0)
            nc.gpsimd.dma_start(sdm1f1[:][1:64], sd1[:][0:63, 2, :, :, :])
            nc.gpsimd.dma_start(sdm1f1[:][64:128], sd1[:][63:127, 2, :, :, :])
            fxp1 = raw1[:][:, :, 1, :, :]
            fyp1 = raw1[:][:, :, 2, :, :]
            # ry planes stored (0,1,3): vconv for ry=0 uses planes 0,1 and
            # the (v-1, ry=3) shift
            vconv(fxp1, fyp1, d1v, s1v, 0, sdm1f1[:][:, 1], d1v[:, 1],
                  sdm1f1[:][:, 0], s1v[:, 1])

            # ---------------- correlation ----------------
            nc.gpsimd.memset(mx[:], float(NEG))

            def f1bc(c, nu, ns):
                return raw1[:][:, 0, c, 0, 2:70].unsqueeze(1).unsqueeze(1) \
                    .broadcast_to([128, nu, ns, GXL])

            def qparams(q):
                return (1, 3) if q == 0 else (0, 4)

            def products(out, src5, s_lo, s_hi, q, ubase, nu):
                ns = s_hi - s_lo
                nc.vector.tensor_tensor(
                    out, f1bc(0, nu, ns), _win(src5, s_lo, s_hi, 0, q,
                                               ubase, nu), Alu.mult)
                pr = prod[:][:, 0:nu, 0:ns, :]
                nc.vector.tensor_tensor(
                    pr, f1bc(1, nu, ns), _win(src5, s_lo, s_hi, 1, q,
                                              ubase, nu), Alu.mult)
                nc.vector.tensor_tensor(out, out, pr, Alu.add)
                nc.vector.tensor_tensor(
                    pr, f1bc(2, nu, ns), _win(src5, s_lo, s_hi, 2, q,
                                              ubase, nu), Alu.mult)
                nc.vector.tensor_tensor(out, out, pr, Alu.add)

            colmax = mx[:][:, 0, :, :]          # [128, 16, GXL], p = 4q+u0
            rowmax = mx[:][:, 1, :, :]          # [128, 16, GXL], slot s

            # early phase: slots 8..11 (ov=0) read feat2 directly,
            # overlapping the f2px replication DMAs
            for q in range(4):
                ubase, nu = qparams(q)
                cE = corrE[:][:, 0:nu, :, :]
                products(cE, raw2[:], 0, 4, q, ubase, nu)
                for u0 in range(nu):
                    nc.vector.tensor_tensor(rowmax[:, 8:12, :],
                                            rowmax[:, 8:12, :],
                                            cE[:, u0, :, :], Alu.max)
                t2 = tA[:][:, 0:nu, 0:2, :]
                nc.vector.tensor_tensor(t2, cE[:, :, 0:2, :],
                                        cE[:, :, 2:4, :], Alu.max)
                nc.vector.tensor_tensor(colmax[:, 4 * q + ubase:4 * q + 4, :],
                                        t2[:, :, 0, :], t2[:, :, 1, :],
                                        Alu.max)

            # late phase: slots 1..7 and 12..15 via f2px
            for q in range(4):
                ubase, nu = qparams(q)
                c7 = corrL[:][:, 0:nu, 1:8, :]
                c4 = corrL[:][:, 0:nu, 12:16, :]
                products(c7, f2px[:], 1, 8, q, ubase, nu)
                products(c4, f2px[:], 12, 16, q, ubase, nu)
                for u0 in range(nu):
                    nc.vector.tensor_tensor(rowmax[:, 1:8, :],
                                            rowmax[:, 1:8, :],
                                            c7[:, u0, :, :], Alu.max)
                    nc.vector.tensor_tensor(rowmax[:, 12:16, :],
                                            rowmax[:, 12:16, :],
                                            c4[:, u0, :, :], Alu.max)
                # colmax tree over slots 1..7
                a3 = tA[:][:, 0:nu, :, :]
                nc.vector.tensor_tensor(a3, c7[:, :, 0:3, :],
                                        c7[:, :, 3:6, :], Alu.max)
                b1 = tB[:][:, 0:nu, :]
                nc.vector.tensor_tensor(b1, a3[:, :, 0, :], a3[:, :, 1, :],
                                        Alu.max)
                nc.vector.tensor_tensor(b1, b1, a3[:, :, 2, :], Alu.max)
                nc.vector.tensor_tensor(b1, b1, c7[:, :, 6, :], Alu.max)
                cm = colmax[:, 4 * q + ubase:4 * q + 4, :]
                nc.vector.tensor_tensor(cm, cm, b1, Alu.max)
                # colmax tree over slots 12..15
                t2 = tA[:][:, 0:nu, 0:2, :]
                nc.vector.tensor_tensor(t2, c4[:, :, 0:2, :],
                                        c4[:, :, 2:4, :], Alu.max)
                nc.vector.tensor_tensor(b1, t2[:, :, 0, :], t2[:, :, 1, :],
                                        Alu.max)
                nc.vector.tensor_tensor(cm, cm, b1, Alu.max)

            # ---------------- batched first-argmax -> grid ----------------
            t8 = tr[:][:, :, 0:8, :]
            nc.vector.tensor_tensor(t8, mx[:][:, :, 0:8, :],
                                    mx[:][:, :, 8:16, :], Alu.max)
            t4 = tr[:][:, :, 0:4, :]
            nc.vector.tensor_tensor(t4, t8[:, :, 0:4, :], t8[:, :, 4:8, :],
                                    Alu.max)
            nc.vector.tensor_tensor(t4[:, :, 0:2, :], t4[:, :, 0:2, :],
                                    t4[:, :, 2:4, :], Alu.max)
            nc.vector.tensor_tensor(m2[:], t4[:, :, 0, :], t4[:, :, 1, :],
                                    Alu.max)
            mb = m2[:].unsqueeze(2).broadcast_to([128, 2, 16, GXL])
            nc.vector.tensor_tensor(wsum[:], mx[:], mb, Alu.is_ge)
            nc.vector.tensor_tensor(wsum[:], wsum[:], wtab, Alu.mult)
            nc.vector.tensor_tensor(t8, wsum[:][:, :, 0:8, :],
                                    wsum[:][:, :, 8:16, :], Alu.max)
            nc.vector.tensor_tensor(t4, t8[:, :, 0:4, :], t8[:, :, 4:8, :],
                                    Alu.max)
            nc.vector.tensor_tensor(t4[:, :, 0:2, :], t4[:, :, 0:2, :],
                                    t4[:, :, 2:4, :], Alu.max)
            nc.vector.tensor_tensor(fm2[:], t4[:, :, 0, :], t4[:, :, 1, :],
                                    Alu.max)
            # disp = (8 - fm)/512 ; zero invalid anchors
            nc.vector.tensor_scalar(fm2[:], fm2[:], -1.0 / 512.0, 8.0 / 512.0,
                                    Alu.mult, Alu.add)
            nc.vector.tensor_tensor(
                grid[:], fm2[:],
                gmask.unsqueeze(1).broadcast_to([128, 2, GXL]), Alu.mult)

            # ---------------- smoothing H-pass (phase weights) -------------
            Wp = _phase_weights()
            hsc2 = hsc[:]
            for p in range(4):
                nc.vector.tensor_scalar_mul(
                    hsc2, grid[:][:, :, 0:64], float(Wp[p, 0]))
                for dd in range(1, 4):
                    nc.vector.scalar_tensor_tensor(
                        hsc2, grid[:][:, :, dd:dd + 64],
                        float(Wp[p, dd]), hsc2, Alu.mult, Alu.add)
                nc.vector.scalar_tensor_tensor(
                    hp[:][:, :, p:256:4], grid[:][:, :, 4:4 + 64],
                    float(Wp[p, 4]), hsc2, Alu.mult, Alu.add)

            # ---------------- V-pass (PE banded matmul) + normalize --------
            nc.vector.tensor_copy(bands2[:], bands)
            nc.scalar.copy(tscr[:][32:33, 0:1], hp[:][32:33, 0, 0:1])
            rhs = hp[:].rearrange("p c x -> p (c x)")
            for t in range(4):
                ps = psum_pool.tile([128, 512], F32, tag="vps")
                nc.tensor.matmul(ps[:], bands2[:][:, t, :], rhs,
                                 start=True, stop=True)
                sm = smsb[t]
                nc.scalar.copy(sm[:].rearrange("p c x -> p (c x)"), ps[:])
                psv = sm[:]
                nq1, nq2, nr, nm = nq1s[t], nq2s[t], nrs[t], nms[t]
                nc.scalar.activation(nq1[:], psv[:, 0, :], Act.Square)
                nc.scalar.activation(nq2[:], psv[:, 1, :], Act.Square)
                nc.vector.scalar_tensor_tensor(nq1[:], nq1[:], 1e-30, nq2[:],
                                               Alu.max, Alu.add)
                nc.scalar.activation(nr[:], nq1[:], Act.Abs_reciprocal_sqrt)
                nc.vector.tensor_tensor(nm[:], nr[:], nr[:], Alu.mult)
                nc.vector.tensor_tensor(nm[:], nm[:], nq1[:], Alu.mult)
                nc.vector.tensor_scalar(nm[:], nm[:], -0.5, 1.5, Alu.mult,
                                        Alu.add)
                nc.vector.tensor_tensor(nr[:], nr[:], nm[:], Alu.mult)
                nc.vector.tensor_tensor(nm[:], nq1[:], nr[:], Alu.mult)
                nc.vector.tensor_scalar(nm[:], nm[:], 1e-6, 1e-6, Alu.max,
                                        Alu.add)
                nc.vector.tensor_tensor(nq2[:], nm[:], nm[:], Alu.mult)
                nc.scalar.activation(nr[:], nq2[:], Act.Abs_reciprocal_sqrt)
                nc.vector.tensor_tensor(nm[:], nr[:], nr[:], Alu.mult)
                nc.vector.tensor_tensor(nm[:], nm[:], nq2[:], Alu.mult)
                nc.vector.tensor_scalar(nm[:], nm[:], -0.5, 1.5, Alu.mult,
                                        Alu.add)
                nc.vector.tensor_tensor(nr[:], nr[:], nm[:], Alu.mult)
                nc.vector.tensor_tensor(outsb[:][:, t, 0, :], psv[:, 0, :],
                                        nr[:], Alu.mult)
                nc.vector.tensor_tensor(outsb[:][:, t, 1, :], psv[:, 1, :],
                                        nr[:], Alu.mult)
            for p0 in range(0, 128, 16):
                nc.sync.dma_start(out_d.ap()[p0:p0 + 16],
                                  outsb[:][p0:p0 + 16])

    nc.compile()
    return nc


_NC_CACHE = None


def _get_nc():
    global _NC_CACHE
    if _NC_CACHE is None:
        _NC_CACHE = build_program()
    return _NC_CACHE


def kernel(frame1, frame2):
    frame1 = np.asarray(frame1, dtype=np.float32)
    frame2 = np.asarray(frame2, dtype=np.float32)
    nc = _get_nc()
    in_maps = _host_inputs(frame1, frame2)
    res = run_bass_kernel_spmd(nc, in_maps, core_ids=list(range(8)))
    if res.exec_time_ns is not None:
        print(f"HW exec time: {res.exec_time_ns} ns")
    out = np.empty((B, 2, H, W), np.float32)
    for b in range(B):
        for w in range(2):
            o = res.results[2 * b + w]["out"]        # [128, 4, 2, 256]
            o = o.transpose(2, 1, 0, 3).reshape(2, H, 256)
            out[b, :, :, 256 * w:256 * w + 256] = o
    return out


# revision 11
# speedup vs baseline: 1.0866x; 1.0866x over previous
"""Dense optical flow kernel for Trainium2, 8-core SPMD.

Pipeline (per core = one (sample, x-half) pair):
  frames (x-polyphase layout q=x%4) -> gray/sobel features in-place
  -> l2-normalize f2 -> 11-slot row-replicated f2px tensor (DMA)
  -> 15x15 windowed correlation in fp32 with ALL operands unit-stride
  (x-polyphase makes every dx shift a contiguous 68-run) -> tree maxes
  -> batched first-argmax -> displacement grid -> separable gaussian
  smoothing (phase H-pass on DVE, banded-matmul V-pass on PE)
  -> direction normalize -> full-res flow.

The x dimension is stored phase-major ([q=x%4][u=x//4]) so the stride-4
anchor/window gathers of the correlation become stride-1 runs; window
shifts dx group by phase q into overlapping-window access patterns
(outer dim stride 1 over an inner stride-1 run) built by AP surgery.
"""

import numpy as np

import concourse.bacc as bacc
import concourse.tile as tile
from concourse import mybir
from concourse.bass_utils import run_bass_kernel_spmd

F32 = mybir.dt.float32
Alu = mybir.AluOpType
Act = mybir.ActivationFunctionType
AX = mybir.AxisListType

H = 512
W = 512
B = 4
XL = 288          # per-core padded column span
U72 = 72          # columns per phase
GXL = 68          # local anchor columns (64 + 2 halo each side)
NEG = np.float32(-1.0e30)
POS = np.float32(3.0e38)


# ----------------------------------------------------------------------------
# constants (host side)
# ----------------------------------------------------------------------------

def _gaussian_sep():
    ax = np.arange(15) - 7
    g = np.exp(-(ax.astype(np.float64) ** 2) / (2.0 * 2.5 ** 2))
    return (g / g.sum())


def _phase_weights():
    g = _gaussian_sep()
    Wp = np.zeros((4, 5), np.float64)
    for p in range(4):
        for t in range(15):
            Wp[p, (p + t - 7) // 4 + 2] += g[t]
    return Wp.astype(np.float32)


def _band_matrices():
    # bands[t][v, y]: out_row(128t+y) = sum_v band[v, y] * hp[v]
    Wp = _phase_weights()
    bands = np.zeros((4, 128, 128), np.float32)
    for t in range(4):
        for y in range(128):
            yg = 128 * t + y
            v0, q = yg // 4, yg % 4
            for d in range(5):
                v = v0 + d - 2
                if 0 <= v < 128:
                    bands[t, v, y] = Wp[q, d]
    return bands


def _phase_major(a):
    # a: (C, 128, 4, XL) -> (128, ry, C, q, u)
    C = a.shape[0]
    return np.ascontiguousarray(
        a.reshape(C, 128, 4, U72, 4).transpose(1, 2, 0, 4, 3))


def _host_inputs(frame1, frame2):
    """Build the 8 per-core input maps."""
    bands = _band_matrices()
    # weight tables for first-argmax (shared across cores)
    wt = np.zeros((2, 16, 1), np.float32)
    for p in range(1, 16):          # ch0: colmax position p = 4q+u0
        q, u0 = p // 4, p % 4
        wt[0, p, 0] = np.float32(16 - 4 * u0 - q)
    for s in range(1, 16):          # ch1: rowmax slot s (i = s-1)
        wt[1, s, 0] = np.float32(16 - s)
    wtab = np.tile(wt, (1, 1, GXL)).reshape(1, 2 * 16 * GXL)
    in_maps = []
    for b in range(B):
        for w in range(2):
            xbase = 256 * w - 16
            sl1 = np.zeros((3, H, XL), np.float32)
            sl2 = np.zeros((3, H, XL), np.float32)
            lo, hi = max(0, xbase), min(W, xbase + XL)
            sl1[:, :, lo - xbase:hi - xbase] = frame1[b][:, :, lo:hi]
            sl2[:, :, lo - xbase:hi - xbase] = frame2[b][:, :, lo:hi]
            il1 = _phase_major(sl1.reshape(3, 128, 4, XL))
            il2 = _phase_major(sl2.reshape(3, 128, 4, XL))
            il1 = np.ascontiguousarray(il1[:, (0, 1, 3)])  # ry 2 unused
            # column-validity mask (phase-major [q, u])
            xcols = xbase + np.arange(XL)
            valid = (xcols >= 0) & (xcols < W)
            xm = np.where(valid, POS, NEG).astype(np.float32)
            xm = np.ascontiguousarray(
                xm.reshape(U72, 4).T).reshape(1, XL)
            # anchor-validity mask
            gxg = 64 * w - 2 + np.arange(GXL)
            gm = ((gxg >= 0) & (gxg < 128)).astype(np.float32)[None, :]
            consts = np.concatenate(
                [np.tile(xm, (128, 1)), np.tile(gm, (128, 1)),
                 bands.transpose(1, 0, 2).reshape(128, 512),
                 np.tile(wtab, (128, 1))], axis=1)
            in_maps.append({"f1s": il1, "f2s": il2,
                            "consts": consts.astype(np.float32)})
    return in_maps


# ----------------------------------------------------------------------------
# device program
# ----------------------------------------------------------------------------

def _win(ap5, s_lo, s_hi, c, q, ubase, nu):
    """Overlapping-window AP: [128, nu, s_hi-s_lo, 68] over a
    [128, S, 3, 4, 72] tensor; dims (u0, slot, gx) strides (1, slotstride, 1)
    reading u = ubase + u0 + gx of phase q, channel c."""
    a = ap5[:, s_lo:s_hi, c, q, ubase:].copy()
    d = a.ap
    d.insert(1, (1, nu))
    d[3] = (1, GXL)
    return a


def _emit_features_ph(nc, raw, sd, R):
    """In-place gray + sobel-H into raw/sd, phase-major layout.

    raw: [128, R, 3, 4, 72] (c0 becomes gray, c1 fx, c2 fy later)
    sd:  [128, R, 2, 4, 72] (idx0 = s, idx1 = d)
    """
    # gray scaled by 1/0.299: uniform feature scale cancels in the
    # per-pixel l2-normalize (sobel is linear), so argmax is unchanged
    g = raw[:][:, :, 0, :, :]                    # [128, R, 4, 72]
    nc.vector.scalar_tensor_tensor(g, raw[:][:, :, 1, :, :],
                                   float(np.float32(0.587 / 0.299)), g,
                                   Alu.mult, Alu.add)
    nc.vector.scalar_tensor_tensor(g, raw[:][:, :, 2, :, :],
                                   float(np.float32(0.114 / 0.299)), g,
                                   Alu.mult, Alu.add)
    s = sd[:][:, :, 0, :, :]
    d = sd[:][:, :, 1, :, :]
    # horizontal sobel, phase-decomposed (x = 4u+q)
    nc.vector.tensor_tensor(d[:, :, 1:3, :], g[:, :, 2:4, :],
                            g[:, :, 0:2, :], Alu.subtract)
    nc.vector.tensor_tensor(d[:, :, 0, 1:U72], g[:, :, 1, 1:U72],
                            g[:, :, 3, 0:U72 - 1], Alu.subtract)
    nc.vector.tensor_tensor(d[:, :, 3, 0:U72 - 1], g[:, :, 0, 1:U72],
                            g[:, :, 2, 0:U72 - 1], Alu.subtract)
    nc.vector.scalar_tensor_tensor(s[:, :, 1:3, :], g[:, :, 1:3, :], 2.0,
                                   g[:, :, 0:2, :], Alu.mult, Alu.add)
    nc.vector.tensor_tensor(s[:, :, 1:3, :], s[:, :, 1:3, :],
                            g[:, :, 2:4, :], Alu.add)
    nc.vector.scalar_tensor_tensor(s[:, :, 0, 1:U72], g[:, :, 0, 1:U72],
                                   2.0, g[:, :, 3, 0:U72 - 1],
                                   Alu.mult, Alu.add)
    nc.vector.tensor_tensor(s[:, :, 0, 1:U72], s[:, :, 0, 1:U72],
                            g[:, :, 1, 1:U72], Alu.add)
    nc.vector.scalar_tensor_tensor(s[:, :, 3, 0:U72 - 1],
                                   g[:, :, 3, 0:U72 - 1], 2.0,
                                   g[:, :, 2, 0:U72 - 1], Alu.mult, Alu.add)
    nc.vector.tensor_tensor(s[:, :, 3, 0:U72 - 1], s[:, :, 3, 0:U72 - 1],
                            g[:, :, 0, 1:U72], Alu.add)
    # boundary columns x=0 (q0,u0) and x=287 (q3,u71): zero s and d
    nc.vector.memset(sd[:][:, :, :, 0, 0:1].squeeze(-1), 0.0)
    nc.vector.memset(sd[:][:, :, :, 3, U72 - 1:U72].squeeze(-1), 0.0)


def build_program():
    nc = bacc.Bacc("TRN2", target_bir_lowering=False, debug=False)

    f1s_d = nc.dram_tensor("f1s", [128, 3, 3, 4, U72], F32,
                           kind="ExternalInput")
    f2s_d = nc.dram_tensor("f2s", [128, 4, 3, 4, U72], F32,
                           kind="ExternalInput")
    NCONST = XL + GXL + 512 + 2 * 16 * GXL
    consts_d = nc.dram_tensor("consts", [128, NCONST], F32,
                              kind="ExternalInput")
    out_d = nc.dram_tensor("out", [128, 4, 2, 256], F32,
                           kind="ExternalOutput")

    with tile.TileContext(nc) as tc:
        with tc.tile_pool(name="main", bufs=1) as pool, \
             tc.tile_pool(name="psum", bufs=4, space="PSUM") as psum_pool:

            raw2 = pool.tile([128, 4, 3, 4, U72], F32)   # becomes feat2
            raw1 = pool.tile([128, 3, 3, 4, U72], F32)   # becomes feat1
            sd2 = pool.tile([128, 4, 2, 4, U72], F32)
            sd1 = pool.tile([128, 3, 2, 4, U72], F32)
            sdm1 = pool.tile([128, 2, 4, U72], F32)
            sdp1 = pool.tile([128, 2, 4, U72], F32)
            sdm1f1 = pool.tile([128, 2, 4, U72], F32)
            consts = pool.tile([128, NCONST], F32)
            q_t = pool.tile([128, 4, 4, U72], F32)
            r0_t = pool.tile([128, 4, 4, U72], F32)
            a_t = pool.tile([128, 4, 4, U72], F32)
            f2px = pool.tile([128, 16, 3, 4, U72], F32)
            # correlation scratch
            corrE = pool.tile([128, 4, 4, GXL], F32)
            corrL = pool.tile([128, 4, 16, GXL], F32, tag="sd2")
            prod = pool.tile([128, 4, 7, GXL], F32, tag="sdp1")
            tA = pool.tile([128, 4, 3, GXL], F32)
            tB = pool.tile([128, 4, GXL], F32)
            mx = pool.tile([128, 2, 16, GXL], F32)
            tr = pool.tile([128, 2, 8, GXL], F32, tag="a_t")
            wsum = pool.tile([128, 2, 16, GXL], F32, tag="sd1")
            m2 = pool.tile([128, 2, GXL], F32)
            fm2 = pool.tile([128, 2, GXL], F32)
            grid = pool.tile([128, 2, GXL], F32)
            hp = pool.tile([128, 2, 256], F32)
            hsc = pool.tile([128, 2, 64], F32)
            tscr = pool.tile([128, 64], F32)
            bands2 = pool.tile([128, 4, 128], F32)
            smsb = [pool.tile([128, 2, 256], F32, name=f"smsb{t}", tag=tg)
                    for t, tg in enumerate(("q_t", "r0_t", "sdm1", "sdm1f1"))]
            nq1s = [pool.tile([128, 256], F32, name=f"nq1_{t}")
                    for t in range(4)]
            nq2s = [pool.tile([128, 256], F32, name=f"nq2_{t}")
                    for t in range(4)]
            nrs = [pool.tile([128, 256], F32, name=f"nr_{t}")
                   for t in range(4)]
            nms = [pool.tile([128, 256], F32, name=f"nm_{t}")
                   for t in range(4)]
            outsb = pool.tile([128, 4, 2, 256], F32, tag="raw1")

            _touch_n = [0]

            def touch(ap):
                # one-wait funnel: absorb a DMA-queue semaphore into the
                # DVE engine clock so consumers carry fewer sync waits
                k = _touch_n[0] = _touch_n[0] + 1
                nc.vector.tensor_copy(tscr[:][32:33, k % 64:k % 64 + 1], ap)

            # ---------------- input DMAs ----------------
            for p0 in range(0, 128, 32):
                nc.sync.dma_start(raw2[:][p0:p0 + 32], f2s_d.ap()[p0:p0 + 32])
            for p0 in range(0, 128, 32):
                nc.sync.dma_start(raw1[:][p0:p0 + 32], f1s_d.ap()[p0:p0 + 32])
            nc.sync.dma_start(consts[:], consts_d.ap())
            touch(consts[:][32:33, 0:1])
            xmask = consts[:][:, 0:XL].rearrange("p (q u) -> p q u", q=4)
            gmask = consts[:][:, XL:XL + GXL]
            bands = consts[:][:, XL + GXL:XL + GXL + 512].rearrange(
                "p (t y) -> p t y", t=4)
            wtab = consts[:][:, XL + GXL + 512:].rearrange(
                "p (c s g) -> p c s g", c=2, s=16)

            # ---------------- frame2 features + normalize ----------------
            _emit_features_ph(nc, raw2, sd2, 4)
            s2v = sd2[:][:, :, 0, :, :]
            d2v = sd2[:][:, :, 1, :, :]
            # cross-partition row shifts for the vertical sobel pass
            nc.vector.memset(sdm1[:][0:1], 0.0)
            nc.gpsimd.dma_start(sdm1[:][1:64], sd2[:][0:63, 3, :, :, :])
            nc.gpsimd.dma_start(sdm1[:][64:128], sd2[:][63:127, 3, :, :, :])
            nc.vector.memset(sdp1[:][96:128], 0.0)
            nc.gpsimd.dma_start(sdp1[:][0:64], sd2[:][1:65, 0, :, :, :])
            nc.gpsimd.dma_start(sdp1[:][64:127], sd2[:][65:128, 0, :, :, :])
            fxp2 = raw2[:][:, :, 1, :, :]
            fyp2 = raw2[:][:, :, 2, :, :]

            def vconv(fxp, fyp, dv, sv, ry, dm1, dp1, sm1, sp1):
                nc.vector.scalar_tensor_tensor(fxp[:, ry], dv[:, ry], 2.0,
                                               dm1, Alu.mult, Alu.add)
                nc.vector.tensor_tensor(fxp[:, ry], fxp[:, ry], dp1, Alu.add)
                nc.vector.tensor_tensor(fyp[:, ry], sp1, sm1, Alu.subtract)

            # ry=1,2 batched (neighbors live inside sd2)
            nc.vector.scalar_tensor_tensor(fxp2[:, 1:3], d2v[:, 1:3], 2.0,
                                           d2v[:, 0:2], Alu.mult, Alu.add)
            nc.vector.tensor_tensor(fxp2[:, 1:3], fxp2[:, 1:3], d2v[:, 2:4],
                                    Alu.add)
            nc.vector.tensor_tensor(fyp2[:, 1:3], s2v[:, 2:4], s2v[:, 0:2],
                                    Alu.subtract)
            vconv(fxp2, fyp2, d2v, s2v, 3, d2v[:, 2], sdp1[:][:, 1],
                  s2v[:, 2], sdp1[:][:, 0])
            vconv(fxp2, fyp2, d2v, s2v, 0, sdm1[:][:, 1], d2v[:, 1],
                  sdm1[:][:, 0], s2v[:, 1])

            # normalize: q = g^2+fx^2+fy^2 (squares on ACT), rsqrt + Newton
            nc.scalar.activation(q_t[:], raw2[:][:, :, 0, :, :], Act.Square)
            nc.scalar.activation(r0_t[:], raw2[:][:, :, 1, :, :], Act.Square)
            nc.scalar.activation(a_t[:], raw2[:][:, :, 2, :, :], Act.Square)
            nc.vector.tensor_tensor(q_t[:], q_t[:], r0_t[:], Alu.add)
            # q = (fy^2 max 1e-24) + (g^2 + fx^2): same zero-pixel guard as
            # max(q, 1e-24) since all terms are >= 0
            nc.vector.scalar_tensor_tensor(q_t[:], a_t[:], 1e-24, q_t[:],
                                           Alu.max, Alu.add)
            nc.scalar.activation(r0_t[:], q_t[:], Act.Abs_reciprocal_sqrt)
            nc.vector.tensor_tensor(a_t[:], r0_t[:], r0_t[:], Alu.mult)
            nc.vector.tensor_tensor(a_t[:], a_t[:], q_t[:], Alu.mult)
            nc.vector.tensor_scalar(a_t[:], a_t[:], -0.5, 1.5, Alu.mult,
                                    Alu.add)
            nc.vector.tensor_tensor(r0_t[:], r0_t[:], a_t[:], Alu.mult)
            for c in range(3):
                nc.vector.tensor_tensor(raw2[:][:, :, c, :, :],
                                        raw2[:][:, :, c, :, :],
                                        r0_t[:], Alu.mult)
            # column-validity mask on the gray plane
            nc.vector.tensor_tensor(
                raw2[:][:, :, 0, :, :], raw2[:][:, :, 0, :, :],
                xmask.unsqueeze(1).broadcast_to([128, 4, 4, U72]), Alu.min)

            # ---------------- f2px replication ----------------
            # out-of-image rows: gray=NEG, fx/fy=0 (overwritten where valid)
            # corner masks on the Pool engine (DVE stays on the main chain)
            f2flat = f2px[:].rearrange("p s c q u -> p s (c q u)")
            for (pa, pb, sa, sb) in ((0, 2, 1, 4), (0, 1, 4, 8),
                                     (96, 128, 12, 16)):
                nc.gpsimd.memset(f2flat[pa:pb, sa:sb, 0:XL], float(NEG))
                nc.gpsimd.memset(f2flat[pa:pb, sa:sb, XL:3 * XL], 0.0)
            # slot s holds rows 4(v+ov)+ry, s = 4*ovi+ovr, ov = ovi-2;
            # ovi=0 pieces ride the SWDGE queues to widen DMA bandwidth
            for (ovi, sa, sb, ra) in ((0, 1, 4, 1), (1, 4, 8, 0),
                                      (3, 12, 16, 0)):
                ov = ovi - 2
                p0, p1 = max(0, -ov), min(128, 128 - ov)
                for q0 in range(0, 128, 32):
                    a, b = max(p0, q0), min(p1, q0 + 32)
                    if a < b:
                        nc.sync.dma_start(
                            f2px[:][a:b, sa:sb, :, :, :],
                            raw2[:][a + ov:b + ov, ra:4, :, :, :])

            # ---------------- frame1 features (anchor rows only) ----------
            _emit_features_ph(nc, raw1, sd1, 3)
            s1v = sd1[:][:, :, 0, :, :]
            d1v = sd1[:][:, :, 1, :, :]
            nc.vector.memset(sdm1f1[:][0:1], 0.0)
            nc.gpsimd.dma_start(sdm1f1[:][1:64], sd1[:][0:63, 2, :, :, :])
            nc.gpsimd.dma_start(sdm1f1[:][64:128], sd1[:][63:127, 2, :, :, :])
            fxp1 = raw1[:][:, :, 1, :, :]
            fyp1 = raw1[:][:, :, 2, :, :]
            # ry planes stored (0,1,3): vconv for ry=0 uses planes 0,1 and
            # the (v-1, ry=3) shift
            vconv(fxp1, fyp1, d1v, s1v, 0, sdm1f1[:][:, 1], d1v[:, 1],
                  sdm1f1[:][:, 0], s1v[:, 1])

            # ---------------- correlation ----------------
            nc.gpsimd.memset(mx[:], float(NEG))

            def f1bc(c, nu, ns):
                return raw1[:][:, 0, c, 0, 2:70].unsqueeze(1).unsqueeze(1) \
                    .broadcast_to([128, nu, ns, GXL])

            def qparams(q):
                return (1, 3) if q == 0 else (0, 4)

            def products(out, src5, s_lo, s_hi, q, ubase, nu):
                ns = s_hi - s_lo
                nc.vector.tensor_tensor(
                    out, f1bc(0, nu, ns), _win(src5, s_lo, s_hi, 0, q,
                                               ubase, nu), Alu.mult)
                pr = prod[:][:, 0:nu, 0:ns, :]
                nc.vector.tensor_tensor(
                    pr, f1bc(1, nu, ns), _win(src5, s_lo, s_hi, 1, q,
                                              ubase, nu), Alu.mult)
                nc.vector.tensor_tensor(out, out, pr, Alu.add)
                nc.vector.tensor_tensor(
                    pr, f1bc(2, nu, ns), _win(src5, s_lo, s_hi, 2, q,
                                              ubase, nu), Alu.mult)
                nc.vector.tensor_tensor(out, out, pr, Alu.add)

            colmax = mx[:][:, 0, :, :]          # [128, 16, GXL], p = 4q+u0
            rowmax = mx[:][:, 1, :, :]          # [128, 16, GXL], slot s

            # early phase: slots 8..11 (ov=0) read feat2 directly,
            # overlapping the f2px replication DMAs
            for q in range(4):
                ubase, nu = qparams(q)
                cE = corrE[:][:, 0:nu, :, :]
                products(cE, raw2[:], 0, 4, q, ubase, nu)
                for u0 in range(nu):
                    nc.vector.tensor_tensor(rowmax[:, 8:12, :],
                                            rowmax[:, 8:12, :],
                                            cE[:, u0, :, :], Alu.max)
                t2 = tA[:][:, 0:nu, 0:2, :]
                nc.vector.tensor_tensor(t2, cE[:, :, 0:2, :],
                                        cE[:, :, 2:4, :], Alu.max)
                nc.vector.tensor_tensor(colmax[:, 4 * q + ubase:4 * q + 4, :],
                                        t2[:, :, 0, :], t2[:, :, 1, :],
                                        Alu.max)

            # late phase: slots 1..7 and 12..15 via f2px
            for q in range(4):
                ubase, nu = qparams(q)
                c7 = corrL[:][:, 0:nu, 1:8, :]
                c4 = corrL[:][:, 0:nu, 12:16, :]
                products(c7, f2px[:], 1, 8, q, ubase, nu)
                products(c4, f2px[:], 12, 16, q, ubase, nu)
                for u0 in range(nu):
                    nc.vector.tensor_tensor(rowmax[:, 1:8, :],
                                            rowmax[:, 1:8, :],
                                            c7[:, u0, :, :], Alu.max)
                    nc.vector.tensor_tensor(rowmax[:, 12:16, :],
                                            rowmax[:, 12:16, :],
                                            c4[:, u0, :, :], Alu.max)
                # colmax tree over slots 1..7
                a3 = tA[:][:, 0:nu, :, :]
                nc.vector.tensor_tensor(a3, c7[:, :, 0:3, :],
                                        c7[:, :, 3:6, :], Alu.max)
                b1 = tB[:][:, 0:nu, :]
                nc.vector.tensor_tensor(b1, a3[:, :, 0, :], a3[:, :, 1, :],
                                        Alu.max)
                nc.vector.tensor_tensor(b1, b1, a3[:, :, 2, :], Alu.max)
                nc.vector.tensor_tensor(b1, b1, c7[:, :, 6, :], Alu.max)
                cm = colmax[:, 4 * q + ubase:4 * q + 4, :]
                nc.vector.tensor_tensor(cm, cm, b1, Alu.max)
                # colmax tree over slots 12..15
                t2 = tA[:][:, 0:nu, 0:2, :]
                nc.vector.tensor_tensor(t2, c4[:, :, 0:2, :],
                                        c4[:, :, 2:4, :], Alu.max)
                nc.vector.tensor_tensor(b1, t2[:, :, 0, :], t2[:, :, 1, :],
                                        Alu.max)
                nc.vector.tensor_tensor(cm, cm, b1, Alu.max)

            # ---------------- batched first-argmax -> grid ----------------
            t8 = tr[:][:, :, 0:8, :]
            nc.vector.tensor_tensor(t8, mx[:][:, :, 0:8, :],
                                    mx[:][:, :, 8:16, :], Alu.max)
            t4 = tr[:][:, :, 0:4, :]
            nc.vector.tensor_tensor(t4, t8[:, :, 0:4, :], t8[:, :, 4:8, :],
                                    Alu.max)
            nc.vector.tensor_tensor(t4[:, :, 0:2, :], t4[:, :, 0:2, :],
                                    t4[:, :, 2:4, :], Alu.max)
            nc.vector.tensor_tensor(m2[:], t4[:, :, 0, :], t4[:, :, 1, :],
                                    Alu.max)
            mb = m2[:].unsqueeze(2).broadcast_to([128, 2, 16, GXL])
            nc.vector.tensor_tensor(wsum[:], mx[:], mb, Alu.is_ge)
            nc.vector.tensor_tensor(wsum[:], wsum[:], wtab, Alu.mult)
            nc.vector.tensor_tensor(t8, wsum[:][:, :, 0:8, :],
                                    wsum[:][:, :, 8:16, :], Alu.max)
            nc.vector.tensor_tensor(t4, t8[:, :, 0:4, :], t8[:, :, 4:8, :],
                                    Alu.max)
            nc.vector.tensor_tensor(t4[:, :, 0:2, :], t4[:, :, 0:2, :],
                                    t4[:, :, 2:4, :], Alu.max)
            nc.vector.tensor_tensor(fm2[:], t4[:, :, 0, :], t4[:, :, 1, :],
                                    Alu.max)
            # disp = (8 - fm)/512 ; zero invalid anchors
            nc.vector.tensor_scalar(fm2[:], fm2[:], -1.0 / 512.0, 8.0 / 512.0,
                                    Alu.mult, Alu.add)
            nc.vector.tensor_tensor(
                grid[:], fm2[:],
                gmask.unsqueeze(1).broadcast_to([128, 2, GXL]), Alu.mult)

            # ---------------- smoothing H-pass (phase weights) -------------
            Wp = _phase_weights()
            hsc2 = hsc[:]
            for p in range(4):
                nc.vector.tensor_scalar_mul(
                    hsc2, grid[:][:, :, 0:64], float(Wp[p, 0]))
                for dd in range(1, 4):
                    nc.vector.scalar_tensor_tensor(
                        hsc2, grid[:][:, :, dd:dd + 64],
                        float(Wp[p, dd]), hsc2, Alu.mult, Alu.add)
                nc.vector.scalar_tensor_tensor(
                    hp[:][:, :, p:256:4], grid[:][:, :, 4:4 + 64],
                    float(Wp[p, 4]), hsc2, Alu.mult, Alu.add)

            # ---------------- V-pass (PE banded matmul) + normalize --------
            nc.vector.tensor_copy(bands2[:], bands)
            nc.scalar.copy(tscr[:][32:33, 0:1], hp[:][32:33, 0, 0:1])
            rhs = hp[:].rearrange("p c x -> p (c x)")
            for t in range(4):
                ps = psum_pool.tile([128, 512], F32, tag="vps")
                nc.tensor.matmul(ps[:], bands2[:][:, t, :], rhs,
                                 start=True, stop=True)
                sm = smsb[t]
                nc.scalar.copy(sm[:].rearrange("p c x -> p (c x)"), ps[:])
                psv = sm[:]
                nq1, nq2, nr, nm = nq1s[t], nq2s[t], nrs[t], nms[t]
                nc.scalar.activation(nq1[:], psv[:, 0, :], Act.Square)
                nc.scalar.activation(nq2[:], psv[:, 1, :], Act.Square)
                nc.vector.scalar_tensor_tensor(nq1[:], nq1[:], 1e-30, nq2[:],
                                               Alu.max, Alu.add)
                nc.scalar.activation(nr[:], nq1[:], Act.Abs_reciprocal_sqrt)
                nc.vector.tensor_tensor(nm[:], nr[:], nr[:], Alu.mult)
                nc.vector.tensor_tensor(nm[:], nm[:], nq1[:], Alu.mult)
                nc.vector.tensor_scalar(nm[:], nm[:], -0.5, 1.5, Alu.mult,
                                        Alu.add)
                nc.vector.tensor_tensor(nr[:], nr[:], nm[:], Alu.mult)
                nc.vector.tensor_tensor(nm[:], nq1[:], nr[:], Alu.mult)
                nc.vector.tensor_scalar(nm[:], nm[:], 1e-6, 1e-6, Alu.max,
                                        Alu.add)
                nc.vector.tensor_tensor(nq2[:], nm[:], nm[:], Alu.mult)
                nc.scalar.activation(nr[:], nq2[:], Act.Abs_reciprocal_sqrt)
                nc.vector.tensor_tensor(nm[:], nr[:], nr[:], Alu.mult)
                nc.vector.tensor_tensor(nm[:], nm[:], nq2[:], Alu.mult)
                nc.vector.tensor_scalar(nm[:], nm[:], -0.5, 1.5, Alu.mult,
                                        Alu.add)
                nc.vector.tensor_tensor(nr[:], nr[:], nm[:], Alu.mult)
                nc.vector.tensor_tensor(outsb[:][:, t, 0, :], psv[:, 0, :],
                                        nr[:], Alu.mult)
                nc.vector.tensor_tensor(outsb[:][:, t, 1, :], psv[:, 1, :],
                                        nr[:], Alu.mult)
            for p0 in range(0, 128, 32):
                nc.sync.dma_start(out_d.ap()[p0:p0 + 32],
                                  outsb[:][p0:p0 + 32])

    nc.compile()
    return nc


_NC_CACHE = None


def _get_nc():
    global _NC_CACHE
    if _NC_CACHE is None:
        _NC_CACHE = build_program()
    return _NC_CACHE


def kernel(frame1, frame2):
    frame1 = np.asarray(frame1, dtype=np.float32)
    frame2 = np.asarray(frame2, dtype=np.float32)
    nc = _get_nc()
    in_maps = _host_inputs(frame1, frame2)
    res = run_bass_kernel_spmd(nc, in_maps, core_ids=list(range(8)))
    if res.exec_time_ns is not None:
        print(f"HW exec time: {res.exec_time_ns} ns")
    out = np.empty((B, 2, H, W), np.float32)
    for b in range(B):
        for w in range(2):
            o = res.results[2 * b + w]["out"]        # [128, 4, 2, 256]
            o = o.transpose(2, 1, 0, 3).reshape(2, H, 256)
            out[b, :, :, 256 * w:256 * w + 256] = o
    return out


# revision 14
# speedup vs baseline: 1.1071x; 1.0189x over previous
"""Dense optical flow kernel for Trainium2, 8-core SPMD.

Pipeline (per core = one (sample, x-half) pair):
  frames (x-polyphase layout q=x%4) -> gray/sobel features in-place
  -> l2-normalize f2 -> 11-slot row-replicated f2px tensor (DMA)
  -> 15x15 windowed correlation in fp32 with ALL operands unit-stride
  (x-polyphase makes every dx shift a contiguous 68-run) -> tree maxes
  -> batched first-argmax -> displacement grid -> separable gaussian
  smoothing (phase H-pass on DVE, banded-matmul V-pass on PE)
  -> direction normalize -> full-res flow.

The x dimension is stored phase-major ([q=x%4][u=x//4]) so the stride-4
anchor/window gathers of the correlation become stride-1 runs; window
shifts dx group by phase q into overlapping-window access patterns
(outer dim stride 1 over an inner stride-1 run) built by AP surgery.
"""

import numpy as np

import concourse.bacc as bacc
import concourse.tile as tile
from concourse import mybir
from concourse.bass_utils import run_bass_kernel_spmd

F32 = mybir.dt.float32
Alu = mybir.AluOpType
Act = mybir.ActivationFunctionType
AX = mybir.AxisListType

H = 512
W = 512
B = 4
XL = 288          # per-core padded column span
U72 = 72          # columns per phase
GXL = 68          # local anchor columns (64 + 2 halo each side)
NEG = np.float32(-1.0e30)
POS = np.float32(3.0e38)


# ----------------------------------------------------------------------------
# constants (host side)
# ----------------------------------------------------------------------------

def _gaussian_sep():
    ax = np.arange(15) - 7
    g = np.exp(-(ax.astype(np.float64) ** 2) / (2.0 * 2.5 ** 2))
    return (g / g.sum())


def _phase_weights():
    g = _gaussian_sep()
    Wp = np.zeros((4, 5), np.float64)
    for p in range(4):
        for t in range(15):
            Wp[p, (p + t - 7) // 4 + 2] += g[t]
    return Wp.astype(np.float32)


def _band_matrices():
    # bands[t][v, y]: out_row(128t+y) = sum_v band[v, y] * hp[v]
    Wp = _phase_weights()
    bands = np.zeros((4, 128, 128), np.float32)
    for t in range(4):
        for y in range(128):
            yg = 128 * t + y
            v0, q = yg // 4, yg % 4
            for d in range(5):
                v = v0 + d - 2
                if 0 <= v < 128:
                    bands[t, v, y] = Wp[q, d]
    return bands


def _phase_major(a):
    # a: (C, 128, 4, XL) -> (128, ry, C, q, u)
    C = a.shape[0]
    return np.ascontiguousarray(
        a.reshape(C, 128, 4, U72, 4).transpose(1, 2, 0, 4, 3))


def _host_inputs(frame1, frame2):
    """Build the 8 per-core input maps."""
    bands = _band_matrices()
    # weight tables for first-argmax (shared across cores)
    wt = np.zeros((2, 16, 1), np.float32)
    for p in range(1, 16):          # ch0: colmax position p = 4q+u0
        q, u0 = p // 4, p % 4
        wt[0, p, 0] = np.float32(16 - 4 * u0 - q)
    for s in range(1, 16):          # ch1: rowmax slot s (i = s-1)
        wt[1, s, 0] = np.float32(16 - s)
    wtab = np.tile(wt, (1, 1, GXL)).reshape(1, 2 * 16 * GXL)
    in_maps = []
    for b in range(B):
        for w in range(2):
            xbase = 256 * w - 16
            sl1 = np.zeros((3, H, XL), np.float32)
            sl2 = np.zeros((3, H, XL), np.float32)
            lo, hi = max(0, xbase), min(W, xbase + XL)
            sl1[:, :, lo - xbase:hi - xbase] = frame1[b][:, :, lo:hi]
            sl2[:, :, lo - xbase:hi - xbase] = frame2[b][:, :, lo:hi]
            il1 = _phase_major(sl1.reshape(3, 128, 4, XL))
            il2 = _phase_major(sl2.reshape(3, 128, 4, XL))
            il1 = np.ascontiguousarray(il1[:, (0, 1, 3)])  # ry 2 unused
            # column-validity mask (phase-major [q, u])
            xcols = xbase + np.arange(XL)
            valid = (xcols >= 0) & (xcols < W)
            xm = np.where(valid, POS, NEG).astype(np.float32)
            xm = np.ascontiguousarray(
                xm.reshape(U72, 4).T).reshape(1, XL)
            # anchor-validity mask
            gxg = 64 * w - 2 + np.arange(GXL)
            gm = ((gxg >= 0) & (gxg < 128)).astype(np.float32)[None, :]
            consts = np.concatenate(
                [np.tile(xm, (128, 1)), np.tile(gm, (128, 1)),
                 bands.transpose(1, 0, 2).reshape(128, 512),
                 np.tile(wtab, (128, 1))], axis=1)
            in_maps.append({"f1s": il1, "f2s": il2,
                            "consts": consts.astype(np.float32)})
    return in_maps


# ----------------------------------------------------------------------------
# device program
# ----------------------------------------------------------------------------

def _win(ap5, s_lo, s_hi, c, q, ubase, nu):
    """Overlapping-window AP: [128, nu, s_hi-s_lo, 68] over a
    [128, S, 3, 4, 72] tensor; dims (u0, slot, gx) strides (1, slotstride, 1)
    reading u = ubase + u0 + gx of phase q, channel c."""
    a = ap5[:, s_lo:s_hi, c, q, ubase:].copy()
    d = a.ap
    d.insert(1, (1, nu))
    d[3] = (1, GXL)
    return a


def _emit_features_ph(nc, raw, sd, R):
    """In-place gray + sobel-H into raw/sd, phase-major layout.

    raw: [128, R, 3, 4, 72] (c0 becomes gray, c1 fx, c2 fy later)
    sd:  [128, R, 2, 4, 72] (idx0 = s, idx1 = d)
    """
    # gray scaled by 1/0.299: uniform feature scale cancels in the
    # per-pixel l2-normalize (sobel is linear), so argmax is unchanged
    g = raw[:][:, :, 0, :, :]                    # [128, R, 4, 72]
    nc.vector.scalar_tensor_tensor(g, raw[:][:, :, 1, :, :],
                                   float(np.float32(0.587 / 0.299)), g,
                                   Alu.mult, Alu.add)
    nc.vector.scalar_tensor_tensor(g, raw[:][:, :, 2, :, :],
                                   float(np.float32(0.114 / 0.299)), g,
                                   Alu.mult, Alu.add)
    s = sd[:][:, :, 0, :, :]
    d = sd[:][:, :, 1, :, :]
    # horizontal sobel, phase-decomposed (x = 4u+q)
    nc.vector.tensor_tensor(d[:, :, 1:3, :], g[:, :, 2:4, :],
                            g[:, :, 0:2, :], Alu.subtract)
    nc.vector.tensor_tensor(d[:, :, 0, 1:U72], g[:, :, 1, 1:U72],
                            g[:, :, 3, 0:U72 - 1], Alu.subtract)
    nc.vector.tensor_tensor(d[:, :, 3, 0:U72 - 1], g[:, :, 0, 1:U72],
                            g[:, :, 2, 0:U72 - 1], Alu.subtract)
    nc.vector.scalar_tensor_tensor(s[:, :, 1:3, :], g[:, :, 1:3, :], 2.0,
                                   g[:, :, 0:2, :], Alu.mult, Alu.add)
    nc.vector.tensor_tensor(s[:, :, 1:3, :], s[:, :, 1:3, :],
                            g[:, :, 2:4, :], Alu.add)
    nc.vector.scalar_tensor_tensor(s[:, :, 0, 1:U72], g[:, :, 0, 1:U72],
                                   2.0, g[:, :, 3, 0:U72 - 1],
                                   Alu.mult, Alu.add)
    nc.vector.tensor_tensor(s[:, :, 0, 1:U72], s[:, :, 0, 1:U72],
                            g[:, :, 1, 1:U72], Alu.add)
    nc.vector.scalar_tensor_tensor(s[:, :, 3, 0:U72 - 1],
                                   g[:, :, 3, 0:U72 - 1], 2.0,
                                   g[:, :, 2, 0:U72 - 1], Alu.mult, Alu.add)
    nc.vector.tensor_tensor(s[:, :, 3, 0:U72 - 1], s[:, :, 3, 0:U72 - 1],
                            g[:, :, 0, 1:U72], Alu.add)
    # boundary columns x=0 (q0,u0) and x=287 (q3,u71): zero s and d
    nc.vector.memset(sd[:][:, :, :, 0, 0:1].squeeze(-1), 0.0)
    nc.vector.memset(sd[:][:, :, :, 3, U72 - 1:U72].squeeze(-1), 0.0)


def build_program():
    nc = bacc.Bacc("TRN2", target_bir_lowering=False, debug=False)

    f1s_d = nc.dram_tensor("f1s", [128, 3, 3, 4, U72], F32,
                           kind="ExternalInput")
    f2s_d = nc.dram_tensor("f2s", [128, 4, 3, 4, U72], F32,
                           kind="ExternalInput")
    NCONST = XL + GXL + 512 + 2 * 16 * GXL
    consts_d = nc.dram_tensor("consts", [128, NCONST], F32,
                              kind="ExternalInput")
    out_d = nc.dram_tensor("out", [128, 4, 2, 256], F32,
                           kind="ExternalOutput")

    with tile.TileContext(nc) as tc:
        with tc.tile_pool(name="main", bufs=1) as pool, \
             tc.tile_pool(name="psum", bufs=4, space="PSUM") as psum_pool:

            raw2 = pool.tile([128, 4, 3, 4, U72], F32)   # becomes feat2
            raw1 = pool.tile([128, 3, 3, 4, U72], F32)   # becomes feat1
            sd2 = pool.tile([128, 4, 2, 4, U72], F32)
            sd1 = pool.tile([128, 3, 2, 4, U72], F32)
            sdm1 = pool.tile([128, 2, 4, U72], F32)
            sdp1 = pool.tile([128, 2, 4, U72], F32)
            sdm1f1 = pool.tile([128, 2, 4, U72], F32)
            consts = pool.tile([128, NCONST], F32)
            q_t = pool.tile([128, 4, 4, U72], F32)
            r0_t = pool.tile([128, 4, 4, U72], F32)
            a_t = pool.tile([128, 4, 4, U72], F32)
            f2px = pool.tile([128, 16, 3, 4, U72], F32)
            # correlation scratch
            corrE = pool.tile([128, 4, 4, GXL], F32)
            corrL = pool.tile([128, 4, 16, GXL], F32, tag="sd2")
            prod = pool.tile([128, 4, 7, GXL], F32, tag="sdp1")
            tA = pool.tile([128, 4, 3, GXL], F32)
            tB = pool.tile([128, 4, GXL], F32)
            mx = pool.tile([128, 2, 16, GXL], F32)
            tr = pool.tile([128, 2, 8, GXL], F32, tag="a_t")
            wsum = pool.tile([128, 2, 16, GXL], F32, tag="sd1")
            m2 = pool.tile([128, 2, GXL], F32)
            fm2 = pool.tile([128, 2, GXL], F32)
            grid = pool.tile([128, 2, GXL], F32)
            hp = pool.tile([128, 2, 256], F32)
            hsc = pool.tile([128, 2, 64], F32)
            tscr = pool.tile([128, 64], F32)
            bands2 = pool.tile([128, 4, 128], F32)
            smsb = [pool.tile([128, 2, 256], F32, name=f"smsb{t}", tag=tg)
                    for t, tg in enumerate(("q_t", "r0_t", "sdm1", "sdm1f1"))]
            nq1s = [pool.tile([128, 256], F32, name=f"nq1_{t}")
                    for t in range(4)]
            nq2s = [pool.tile([128, 256], F32, name=f"nq2_{t}")
                    for t in range(4)]
            nrs = [pool.tile([128, 256], F32, name=f"nr_{t}")
                   for t in range(4)]
            nms = [pool.tile([128, 256], F32, name=f"nm_{t}")
                   for t in range(4)]
            outsb = pool.tile([128, 4, 2, 256], F32, tag="raw1")

            _touch_n = [0]

            def touch(ap):
                # one-wait funnel: absorb a DMA-queue semaphore into the
                # DVE engine clock so consumers carry fewer sync waits
                k = _touch_n[0] = _touch_n[0] + 1
                nc.vector.tensor_copy(tscr[:][32:33, k % 64:k % 64 + 1], ap)

            # ---------------- input DMAs ----------------
            # f2s first and widest (feature chain blocks on all of it)
            for p0 in range(0, 128, 16):
                nc.sync.dma_start(raw2[:][p0:p0 + 16], f2s_d.ap()[p0:p0 + 16])
            for p0 in range(0, 128, 32):
                nc.sync.dma_start(raw1[:][p0:p0 + 32], f1s_d.ap()[p0:p0 + 32])
            # consts: early piece (masks+bands) separate from the argmax
            # weight table so the big table stays off the critical path
            NC0 = XL + GXL + 512
            nc.sync.dma_start(consts[:][:, 0:NC0], consts_d.ap()[:, 0:NC0])
            nc.sync.dma_start(consts[:][:, NC0:], consts_d.ap()[:, NC0:])
            touch(consts[:][32:33, 0:1])
            xmask = consts[:][:, 0:XL].rearrange("p (q u) -> p q u", q=4)
            gmask = consts[:][:, XL:XL + GXL]
            bands = consts[:][:, XL + GXL:XL + GXL + 512].rearrange(
                "p (t y) -> p t y", t=4)
            wtab = consts[:][:, XL + GXL + 512:].rearrange(
                "p (c s g) -> p c s g", c=2, s=16)

            # ---------------- frame2 features + normalize ----------------
            _emit_features_ph(nc, raw2, sd2, 4)
            s2v = sd2[:][:, :, 0, :, :]
            d2v = sd2[:][:, :, 1, :, :]
            # cross-partition row shifts for the vertical sobel pass
            nc.vector.memset(sdm1[:][0:1], 0.0)
            nc.gpsimd.dma_start(sdm1[:][1:64], sd2[:][0:63, 3, :, :, :])
            nc.gpsimd.dma_start(sdm1[:][64:128], sd2[:][63:127, 3, :, :, :])
            nc.vector.memset(sdp1[:][96:128], 0.0)
            nc.gpsimd.dma_start(sdp1[:][0:64], sd2[:][1:65, 0, :, :, :])
            nc.gpsimd.dma_start(sdp1[:][64:127], sd2[:][65:128, 0, :, :, :])
            fxp2 = raw2[:][:, :, 1, :, :]
            fyp2 = raw2[:][:, :, 2, :, :]

            def vconv(fxp, fyp, dv, sv, ry, dm1, dp1, sm1, sp1):
                nc.vector.scalar_tensor_tensor(fxp[:, ry], dv[:, ry], 2.0,
                                               dm1, Alu.mult, Alu.add)
                nc.vector.tensor_tensor(fxp[:, ry], fxp[:, ry], dp1, Alu.add)
                nc.vector.tensor_tensor(fyp[:, ry], sp1, sm1, Alu.subtract)

            # ry=1,2 batched (neighbors live inside sd2)
            nc.vector.scalar_tensor_tensor(fxp2[:, 1:3], d2v[:, 1:3], 2.0,
                                           d2v[:, 0:2], Alu.mult, Alu.add)
            nc.vector.tensor_tensor(fxp2[:, 1:3], fxp2[:, 1:3], d2v[:, 2:4],
                                    Alu.add)
            nc.vector.tensor_tensor(fyp2[:, 1:3], s2v[:, 2:4], s2v[:, 0:2],
                                    Alu.subtract)
            vconv(fxp2, fyp2, d2v, s2v, 3, d2v[:, 2], sdp1[:][:, 1],
                  s2v[:, 2], sdp1[:][:, 0])
            vconv(fxp2, fyp2, d2v, s2v, 0, sdm1[:][:, 1], d2v[:, 1],
                  sdm1[:][:, 0], s2v[:, 1])

            # normalize: q = g^2+fx^2+fy^2 (squares on ACT), rsqrt + Newton
            nc.scalar.activation(q_t[:], raw2[:][:, :, 0, :, :], Act.Square)
            nc.scalar.activation(r0_t[:], raw2[:][:, :, 1, :, :], Act.Square)
            nc.scalar.activation(a_t[:], raw2[:][:, :, 2, :, :], Act.Square)
            nc.vector.tensor_tensor(q_t[:], q_t[:], r0_t[:], Alu.add)
            # q = (fy^2 max 1e-24) + (g^2 + fx^2): same zero-pixel guard as
            # max(q, 1e-24) since all terms are >= 0
            nc.vector.scalar_tensor_tensor(q_t[:], a_t[:], 1e-24, q_t[:],
                                           Alu.max, Alu.add)
            nc.scalar.activation(r0_t[:], q_t[:], Act.Abs_reciprocal_sqrt)
            nc.vector.tensor_tensor(a_t[:], r0_t[:], r0_t[:], Alu.mult)
            nc.vector.tensor_tensor(a_t[:], a_t[:], q_t[:], Alu.mult)
            nc.vector.tensor_scalar(a_t[:], a_t[:], -0.5, 1.5, Alu.mult,
                                    Alu.add)
            nc.vector.tensor_tensor(r0_t[:], r0_t[:], a_t[:], Alu.mult)
            for c in range(3):
                nc.vector.tensor_tensor(raw2[:][:, :, c, :, :],
                                        raw2[:][:, :, c, :, :],
                                        r0_t[:], Alu.mult)
            # column-validity mask on the gray plane
            nc.vector.tensor_tensor(
                raw2[:][:, :, 0, :, :], raw2[:][:, :, 0, :, :],
                xmask.unsqueeze(1).broadcast_to([128, 4, 4, U72]), Alu.min)

            # ---------------- f2px replication ----------------
            # out-of-image rows: gray=NEG, fx/fy=0 (overwritten where valid)
            # corner masks on the Pool engine (DVE stays on the main chain)
            f2flat = f2px[:].rearrange("p s c q u -> p s (c q u)")
            for (pa, pb, sa, sb) in ((0, 2, 1, 4), (0, 1, 4, 8),
                                     (96, 128, 12, 16)):
                nc.gpsimd.memset(f2flat[pa:pb, sa:sb, 0:XL], float(NEG))
                nc.gpsimd.memset(f2flat[pa:pb, sa:sb, XL:3 * XL], 0.0)
            # slot s holds rows 4(v+ov)+ry, s = 4*ovi+ovr, ov = ovi-2;
            # ovi=0 pieces ride the SWDGE queues to widen DMA bandwidth
            for (ovi, sa, sb, ra) in ((0, 1, 4, 1), (1, 4, 8, 0),
                                      (3, 12, 16, 0)):
                ov = ovi - 2
                p0, p1 = max(0, -ov), min(128, 128 - ov)
                for q0 in range(0, 128, 16):
                    a, b = max(p0, q0), min(p1, q0 + 16)
                    if a < b:
                        nc.sync.dma_start(
                            f2px[:][a:b, sa:sb, :, :, :],
                            raw2[:][a + ov:b + ov, ra:4, :, :, :])

            # ---------------- frame1 features (anchor rows only) ----------
            _emit_features_ph(nc, raw1, sd1, 3)
            s1v = sd1[:][:, :, 0, :, :]
            d1v = sd1[:][:, :, 1, :, :]
            nc.vector.memset(sdm1f1[:][0:1], 0.0)
            nc.gpsimd.dma_start(sdm1f1[:][1:64], sd1[:][0:63, 2, :, :, :])
            nc.gpsimd.dma_start(sdm1f1[:][64:128], sd1[:][63:127, 2, :, :, :])
            fxp1 = raw1[:][:, :, 1, :, :]
            fyp1 = raw1[:][:, :, 2, :, :]
            # ry planes stored (0,1,3): vconv for ry=0 uses planes 0,1 and
            # the (v-1, ry=3) shift
            vconv(fxp1, fyp1, d1v, s1v, 0, sdm1f1[:][:, 1], d1v[:, 1],
                  sdm1f1[:][:, 0], s1v[:, 1])

            # ---------------- correlation ----------------
            nc.gpsimd.memset(mx[:], float(NEG))

            def f1bc(c, nu, ns):
                return raw1[:][:, 0, c, 0, 2:70].unsqueeze(1).unsqueeze(1) \
                    .broadcast_to([128, nu, ns, GXL])

            def qparams(q):
                return (1, 3) if q == 0 else (0, 4)

            def products(out, src5, s_lo, s_hi, q, ubase, nu):
                ns = s_hi - s_lo
                nc.vector.tensor_tensor(
                    out, f1bc(0, nu, ns), _win(src5, s_lo, s_hi, 0, q,
                                               ubase, nu), Alu.mult)
                pr = prod[:][:, 0:nu, 0:ns, :]
                nc.vector.tensor_tensor(
                    pr, f1bc(1, nu, ns), _win(src5, s_lo, s_hi, 1, q,
                                              ubase, nu), Alu.mult)
                nc.vector.tensor_tensor(out, out, pr, Alu.add)
                nc.vector.tensor_tensor(
                    pr, f1bc(2, nu, ns), _win(src5, s_lo, s_hi, 2, q,
                                              ubase, nu), Alu.mult)
                nc.vector.tensor_tensor(out, out, pr, Alu.add)

            colmax = mx[:][:, 0, :, :]          # [128, 16, GXL], p = 4q+u0
            rowmax = mx[:][:, 1, :, :]          # [128, 16, GXL], slot s

            # early phase: slots 8..11 (ov=0) read feat2 directly,
            # overlapping the f2px replication DMAs
            for q in range(4):
                ubase, nu = qparams(q)
                cE = corrE[:][:, 0:nu, :, :]
                products(cE, raw2[:], 0, 4, q, ubase, nu)
                for u0 in range(nu):
                    nc.vector.tensor_tensor(rowmax[:, 8:12, :],
                                            rowmax[:, 8:12, :],
                                            cE[:, u0, :, :], Alu.max)
                t2 = tA[:][:, 0:nu, 0:2, :]
                nc.vector.tensor_tensor(t2, cE[:, :, 0:2, :],
                                        cE[:, :, 2:4, :], Alu.max)
                nc.vector.tensor_tensor(colmax[:, 4 * q + ubase:4 * q + 4, :],
                                        t2[:, :, 0, :], t2[:, :, 1, :],
                                        Alu.max)

            # late phase: slots 1..7 and 12..15 via f2px
            for q in range(4):
                ubase, nu = qparams(q)
                c7 = corrL[:][:, 0:nu, 1:8, :]
                c4 = corrL[:][:, 0:nu, 12:16, :]
                products(c7, f2px[:], 1, 8, q, ubase, nu)
                products(c4, f2px[:], 12, 16, q, ubase, nu)
                for u0 in range(nu):
                    nc.vector.tensor_tensor(rowmax[:, 1:8, :],
                                            rowmax[:, 1:8, :],
                                            c7[:, u0, :, :], Alu.max)
                    nc.vector.tensor_tensor(rowmax[:, 12:16, :],
                                            rowmax[:, 12:16, :],
                                            c4[:, u0, :, :], Alu.max)
                # colmax tree over slots 1..7
                a3 = tA[:][:, 0:nu, :, :]
                nc.vector.tensor_tensor(a3, c7[:, :, 0:3, :],
                                        c7[:, :, 3:6, :], Alu.max)
                b1 = tB[:][:, 0:nu, :]
                nc.vector.tensor_tensor(b1, a3[:, :, 0, :], a3[:, :, 1, :],
                                        Alu.max)
                nc.vector.tensor_tensor(b1, b1, a3[:, :, 2, :], Alu.max)
                nc.vector.tensor_tensor(b1, b1, c7[:, :, 6, :], Alu.max)
                cm = colmax[:, 4 * q + ubase:4 * q + 4, :]
                nc.vector.tensor_tensor(cm, cm, b1, Alu.max)
                # colmax tree over slots 12..15
                t2 = tA[:][:, 0:nu, 0:2, :]
                nc.vector.tensor_tensor(t2, c4[:, :, 0:2, :],
                                        c4[:, :, 2:4, :], Alu.max)
                nc.vector.tensor_tensor(b1, t2[:, :, 0, :], t2[:, :, 1, :],
                                        Alu.max)
                nc.vector.tensor_tensor(cm, cm, b1, Alu.max)

            # ---------------- batched first-argmax -> grid ----------------
            t8 = tr[:][:, :, 0:8, :]
            nc.vector.tensor_tensor(t8, mx[:][:, :, 0:8, :],
                                    mx[:][:, :, 8:16, :], Alu.max)
            t4 = tr[:][:, :, 0:4, :]
            nc.vector.tensor_tensor(t4, t8[:, :, 0:4, :], t8[:, :, 4:8, :],
                                    Alu.max)
            nc.vector.tensor_tensor(t4[:, :, 0:2, :], t4[:, :, 0:2, :],
                                    t4[:, :, 2:4, :], Alu.max)
            nc.vector.tensor_tensor(m2[:], t4[:, :, 0, :], t4[:, :, 1, :],
                                    Alu.max)
            mb = m2[:].unsqueeze(2).broadcast_to([128, 2, 16, GXL])
            nc.vector.tensor_tensor(wsum[:], mx[:], mb, Alu.is_ge)
            nc.vector.tensor_tensor(wsum[:], wsum[:], wtab, Alu.mult)
            nc.vector.tensor_tensor(t8, wsum[:][:, :, 0:8, :],
                                    wsum[:][:, :, 8:16, :], Alu.max)
            nc.vector.tensor_tensor(t4, t8[:, :, 0:4, :], t8[:, :, 4:8, :],
                                    Alu.max)
            nc.vector.tensor_tensor(t4[:, :, 0:2, :], t4[:, :, 0:2, :],
                                    t4[:, :, 2:4, :], Alu.max)
            nc.vector.tensor_tensor(fm2[:], t4[:, :, 0, :], t4[:, :, 1, :],
                                    Alu.max)
            # disp = (8 - fm)/512 ; zero invalid anchors
            nc.vector.tensor_scalar(fm2[:], fm2[:], -1.0 / 512.0, 8.0 / 512.0,
                                    Alu.mult, Alu.add)
            nc.vector.tensor_tensor(
                grid[:], fm2[:],
                gmask.unsqueeze(1).broadcast_to([128, 2, GXL]), Alu.mult)

            # ---------------- smoothing H-pass (phase weights) -------------
            Wp = _phase_weights()
            hsc2 = hsc[:]
            for p in range(4):
                nc.vector.tensor_scalar_mul(
                    hsc2, grid[:][:, :, 0:64], float(Wp[p, 0]))
                for dd in range(1, 4):
                    nc.vector.scalar_tensor_tensor(
                        hsc2, grid[:][:, :, dd:dd + 64],
                        float(Wp[p, dd]), hsc2, Alu.mult, Alu.add)
                nc.vector.scalar_tensor_tensor(
                    hp[:][:, :, p:256:4], grid[:][:, :, 4:4 + 64],
                    float(Wp[p, 4]), hsc2, Alu.mult, Alu.add)

            # ---------------- V-pass (PE banded matmul) + normalize --------
            nc.vector.tensor_copy(bands2[:], bands)
            nc.scalar.copy(tscr[:][32:33, 0:1], hp[:][32:33, 0, 0:1])
            rhs = hp[:].rearrange("p c x -> p (c x)")
            for t in range(4):
                ps = psum_pool.tile([128, 512], F32, tag="vps")
                nc.tensor.matmul(ps[:], bands2[:][:, t, :], rhs,
                                 start=True, stop=True)
                sm = smsb[t]
                nc.scalar.copy(sm[:].rearrange("p c x -> p (c x)"), ps[:])
                psv = sm[:]
                nq1, nq2, nr, nm = nq1s[t], nq2s[t], nrs[t], nms[t]
                nc.scalar.activation(nq1[:], psv[:, 0, :], Act.Square)
                nc.scalar.activation(nq2[:], psv[:, 1, :], Act.Square)
                nc.vector.scalar_tensor_tensor(nq1[:], nq1[:], 1e-30, nq2[:],
                                               Alu.max, Alu.add)
                # mag = q*rsqrt(q); d = max(mag,1e-6)+1e-6; out = flow/d
                # (raw ACT rsqrt, ~5e-4 rel err: well under tolerance)
                nc.scalar.activation(nr[:], nq1[:], Act.Abs_reciprocal_sqrt)
                nc.vector.tensor_tensor(nm[:], nq1[:], nr[:], Alu.mult)
                nc.vector.tensor_scalar(nm[:], nm[:], 1e-6, 1e-6, Alu.max,
                                        Alu.add)
                nc.vector.tensor_tensor(nq2[:], nm[:], nm[:], Alu.mult)
                nc.scalar.activation(nr[:], nq2[:], Act.Abs_reciprocal_sqrt)
                nc.vector.tensor_tensor(outsb[:][:, t, 0, :], psv[:, 0, :],
                                        nr[:], Alu.mult)
                nc.vector.tensor_tensor(outsb[:][:, t, 1, :], psv[:, 1, :],
                                        nr[:], Alu.mult)
                # stream this t-plane out while later planes still compute
                for p0 in range(0, 128, 32):
                    nc.sync.dma_start(out_d.ap()[p0:p0 + 32, t],
                                      outsb[:][p0:p0 + 32, t])

    nc.compile()
    return nc


_NC_CACHE = None


def _get_nc():
    global _NC_CACHE
    if _NC_CACHE is None:
        _NC_CACHE = build_program()
    return _NC_CACHE


def kernel(frame1, frame2):
    frame1 = np.asarray(frame1, dtype=np.float32)
    frame2 = np.asarray(frame2, dtype=np.float32)
    nc = _get_nc()
    in_maps = _host_inputs(frame1, frame2)
    res = run_bass_kernel_spmd(nc, in_maps, core_ids=list(range(8)))
    if res.exec_time_ns is not None:
        print(f"HW exec time: {res.exec_time_ns} ns")
    out = np.empty((B, 2, H, W), np.float32)
    for b in range(B):
        for w in range(2):
            o = res.results[2 * b + w]["out"]        # [128, 4, 2, 256]
            o = o.transpose(2, 1, 0, 3).reshape(2, H, 256)
            out[b, :, :, 256 * w:256 * w + 256] = o
    return out


# revision 16
# speedup vs baseline: 1.1465x; 1.0355x over previous
"""Dense optical flow kernel for Trainium2, 8-core SPMD.

Pipeline (per core = one (sample, x-half) pair):
  frames (x-polyphase layout q=x%4) -> gray/sobel features in-place
  -> l2-normalize f2 -> 11-slot row-replicated f2px tensor (DMA)
  -> 15x15 windowed correlation in fp32 with ALL operands unit-stride
  (x-polyphase makes every dx shift a contiguous 68-run) -> tree maxes
  -> batched first-argmax -> displacement grid -> separable gaussian
  smoothing (phase H-pass on DVE, banded-matmul V-pass on PE)
  -> direction normalize -> full-res flow.

The x dimension is stored phase-major ([q=x%4][u=x//4]) so the stride-4
anchor/window gathers of the correlation become stride-1 runs; window
shifts dx group by phase q into overlapping-window access patterns
(outer dim stride 1 over an inner stride-1 run) built by AP surgery.
"""

import numpy as np

import concourse.bacc as bacc
import concourse.tile as tile
from concourse import mybir
from concourse.bass_utils import run_bass_kernel_spmd

F32 = mybir.dt.float32
Alu = mybir.AluOpType
Act = mybir.ActivationFunctionType
AX = mybir.AxisListType

H = 512
W = 512
B = 4
XL = 288          # per-core padded column span
U72 = 72          # columns per phase
GXL = 68          # local anchor columns (64 + 2 halo each side)
NEG = np.float32(-1.0e30)
POS = np.float32(3.0e38)


# ----------------------------------------------------------------------------
# constants (host side)
# ----------------------------------------------------------------------------

def _gaussian_sep():
    ax = np.arange(15) - 7
    g = np.exp(-(ax.astype(np.float64) ** 2) / (2.0 * 2.5 ** 2))
    return (g / g.sum())


def _phase_weights():
    g = _gaussian_sep()
    Wp = np.zeros((4, 5), np.float64)
    for p in range(4):
        for t in range(15):
            Wp[p, (p + t - 7) // 4 + 2] += g[t]
    return Wp.astype(np.float32)


def _band_matrices():
    # bands[t][v, y]: out_row(128t+y) = sum_v band[v, y] * hp[v]
    Wp = _phase_weights()
    bands = np.zeros((4, 128, 128), np.float32)
    for t in range(4):
        for y in range(128):
            yg = 128 * t + y
            v0, q = yg // 4, yg % 4
            for d in range(5):
                v = v0 + d - 2
                if 0 <= v < 128:
                    bands[t, v, y] = Wp[q, d]
    return bands


def _phase_major(a):
    # a: (C, 128, 4, XL) -> (128, ry, C, q, u)
    C = a.shape[0]
    return np.ascontiguousarray(
        a.reshape(C, 128, 4, U72, 4).transpose(1, 2, 0, 4, 3))


def _host_inputs(frame1, frame2):
    """Build the 8 per-core input maps."""
    bands = _band_matrices()
    # weight tables for first-argmax (shared across cores)
    wt = np.zeros((2, 16, 1), np.float32)
    for p in range(1, 16):          # ch0: colmax position p = 4q+u0
        q, u0 = p // 4, p % 4
        wt[0, p, 0] = np.float32(16 - 4 * u0 - q)
    for s in range(1, 16):          # ch1: rowmax slot s (i = s-1)
        wt[1, s, 0] = np.float32(16 - s)
    wtab = np.tile(wt, (1, 1, GXL)).reshape(1, 2 * 16 * GXL)
    in_maps = []
    for b in range(B):
        for w in range(2):
            xbase = 256 * w - 16
            sl1 = np.zeros((3, H, XL), np.float32)
            sl2 = np.zeros((3, H, XL), np.float32)
            lo, hi = max(0, xbase), min(W, xbase + XL)
            sl1[:, :, lo - xbase:hi - xbase] = frame1[b][:, :, lo:hi]
            sl2[:, :, lo - xbase:hi - xbase] = frame2[b][:, :, lo:hi]
            il1 = _phase_major(sl1.reshape(3, 128, 4, XL))
            il2 = _phase_major(sl2.reshape(3, 128, 4, XL))
            il1 = np.ascontiguousarray(il1[:, (0, 1, 3)])  # ry 2 unused
            # column-validity mask (phase-major [q, u])
            xcols = xbase + np.arange(XL)
            valid = (xcols >= 0) & (xcols < W)
            xm = np.where(valid, POS, NEG).astype(np.float32)
            xm = np.ascontiguousarray(
                xm.reshape(U72, 4).T).reshape(1, XL)
            # anchor-validity mask
            gxg = 64 * w - 2 + np.arange(GXL)
            gm = ((gxg >= 0) & (gxg < 128)).astype(np.float32)[None, :]
            consts = np.concatenate(
                [np.tile(xm, (128, 1)), np.tile(gm, (128, 1)),
                 bands.transpose(1, 0, 2).reshape(128, 512),
                 np.tile(wtab, (128, 1))], axis=1)
            in_maps.append({"f1s": il1, "f2s": il2,
                            "consts": consts.astype(np.float32)})
    return in_maps


# ----------------------------------------------------------------------------
# device program
# ----------------------------------------------------------------------------

def _win(ap5, s_lo, s_hi, c, q, ubase, nu):
    """Overlapping-window AP: [128, nu, s_hi-s_lo, 68] over a
    [128, S, 3, 4, 72] tensor; dims (u0, slot, gx) strides (1, slotstride, 1)
    reading u = ubase + u0 + gx of phase q, channel c."""
    a = ap5[:, s_lo:s_hi, c, q, ubase:].copy()
    d = a.ap
    d.insert(1, (1, nu))
    d[3] = (1, GXL)
    return a


def _emit_features_ph(nc, raw, sd, R):
    """In-place gray + sobel-H into raw/sd, phase-major layout.

    raw: [128, R, 3, 4, 72] (c0 becomes gray, c1 fx, c2 fy later)
    sd:  [128, R, 2, 4, 72] (idx0 = s, idx1 = d)
    """
    # gray scaled by 1/0.299: uniform feature scale cancels in the
    # per-pixel l2-normalize (sobel is linear), so argmax is unchanged
    g = raw[:][:, :, 0, :, :]                    # [128, R, 4, 72]
    nc.vector.scalar_tensor_tensor(g, raw[:][:, :, 1, :, :],
                                   float(np.float32(0.587 / 0.299)), g,
                                   Alu.mult, Alu.add)
    nc.vector.scalar_tensor_tensor(g, raw[:][:, :, 2, :, :],
                                   float(np.float32(0.114 / 0.299)), g,
                                   Alu.mult, Alu.add)
    s = sd[:][:, :, 0, :, :]
    d = sd[:][:, :, 1, :, :]
    # horizontal sobel, phase-decomposed (x = 4u+q)
    nc.vector.tensor_tensor(d[:, :, 1:3, :], g[:, :, 2:4, :],
                            g[:, :, 0:2, :], Alu.subtract)
    nc.vector.tensor_tensor(d[:, :, 0, 1:U72], g[:, :, 1, 1:U72],
                            g[:, :, 3, 0:U72 - 1], Alu.subtract)
    nc.vector.tensor_tensor(d[:, :, 3, 0:U72 - 1], g[:, :, 0, 1:U72],
                            g[:, :, 2, 0:U72 - 1], Alu.subtract)
    nc.vector.scalar_tensor_tensor(s[:, :, 1:3, :], g[:, :, 1:3, :], 2.0,
                                   g[:, :, 0:2, :], Alu.mult, Alu.add)
    nc.vector.tensor_tensor(s[:, :, 1:3, :], s[:, :, 1:3, :],
                            g[:, :, 2:4, :], Alu.add)
    nc.vector.scalar_tensor_tensor(s[:, :, 0, 1:U72], g[:, :, 0, 1:U72],
                                   2.0, g[:, :, 3, 0:U72 - 1],
                                   Alu.mult, Alu.add)
    nc.vector.tensor_tensor(s[:, :, 0, 1:U72], s[:, :, 0, 1:U72],
                            g[:, :, 1, 1:U72], Alu.add)
    nc.vector.scalar_tensor_tensor(s[:, :, 3, 0:U72 - 1],
                                   g[:, :, 3, 0:U72 - 1], 2.0,
                                   g[:, :, 2, 0:U72 - 1], Alu.mult, Alu.add)
    nc.vector.tensor_tensor(s[:, :, 3, 0:U72 - 1], s[:, :, 3, 0:U72 - 1],
                            g[:, :, 0, 1:U72], Alu.add)
    # boundary columns x=0 (q0,u0) and x=287 (q3,u71): zero s and d
    nc.vector.memset(sd[:][:, :, :, 0, 0:1].squeeze(-1), 0.0)
    nc.vector.memset(sd[:][:, :, :, 3, U72 - 1:U72].squeeze(-1), 0.0)


def build_program():
    nc = bacc.Bacc("TRN2", target_bir_lowering=False, debug=False)

    f1s_d = nc.dram_tensor("f1s", [128, 3, 3, 4, U72], F32,
                           kind="ExternalInput")
    f2s_d = nc.dram_tensor("f2s", [128, 4, 3, 4, U72], F32,
                           kind="ExternalInput")
    NCONST = XL + GXL + 512 + 2 * 16 * GXL
    consts_d = nc.dram_tensor("consts", [128, NCONST], F32,
                              kind="ExternalInput")
    out_d = nc.dram_tensor("out", [128, 4, 2, 256], F32,
                           kind="ExternalOutput")

    with tile.TileContext(nc) as tc:
        with tc.tile_pool(name="main", bufs=1) as pool, \
             tc.tile_pool(name="psum", bufs=4, space="PSUM") as psum_pool:

            raw2 = pool.tile([128, 4, 3, 4, U72], F32)   # becomes feat2
            raw1 = pool.tile([128, 3, 3, 4, U72], F32)   # becomes feat1
            sd2 = pool.tile([128, 4, 2, 4, U72], F32)
            sd1 = pool.tile([128, 3, 2, 4, U72], F32)
            sdm1 = pool.tile([128, 2, 4, U72], F32)
            sdp1 = pool.tile([128, 2, 4, U72], F32)
            sdm1f1 = pool.tile([128, 2, 4, U72], F32)
            consts = pool.tile([128, NCONST], F32)
            q_t = pool.tile([128, 4, 4, U72], F32)
            r0_t = pool.tile([128, 4, 4, U72], F32)
            a_t = pool.tile([128, 4, 4, U72], F32)
            f2px = pool.tile([128, 16, 3, 4, U72], F32)
            # correlation scratch
            corrE = pool.tile([128, 4, 4, GXL], F32)
            corrL = pool.tile([128, 4, 16, GXL], F32, tag="sd2")
            prod = pool.tile([128, 4, 7, GXL], F32, tag="sdp1")
            tA = pool.tile([128, 4, 3, GXL], F32)
            tB = pool.tile([128, 4, GXL], F32)
            mx = pool.tile([128, 2, 16, GXL], F32)
            tr = pool.tile([128, 2, 8, GXL], F32, tag="a_t")
            wsum = pool.tile([128, 2, 16, GXL], F32, tag="sd1")
            m2 = pool.tile([128, 2, GXL], F32)
            fm2 = pool.tile([128, 2, GXL], F32)
            grid = pool.tile([128, 2, GXL], F32)
            hp = pool.tile([128, 2, 256], F32)
            hsc = pool.tile([128, 2, 64], F32)
            tscr = pool.tile([128, 64], F32)
            bands2 = pool.tile([128, 4, 128], F32)
            smsb = [pool.tile([128, 2, 256], F32, name=f"smsb{t}", tag=tg)
                    for t, tg in enumerate(("q_t", "r0_t", "sdm1", "sdm1f1"))]
            nq1s = [pool.tile([128, 256], F32, name=f"nq1_{t}")
                    for t in range(4)]
            nq2s = [pool.tile([128, 256], F32, name=f"nq2_{t}")
                    for t in range(4)]
            nrs = [pool.tile([128, 256], F32, name=f"nr_{t}")
                   for t in range(4)]
            nms = [pool.tile([128, 256], F32, name=f"nm_{t}")
                   for t in range(4)]
            outsb = pool.tile([128, 4, 2, 256], F32, tag="raw1")

            _touch_n = [0]

            def touch(ap):
                # one-wait funnel: absorb a DMA-queue semaphore into the
                # DVE engine clock so consumers carry fewer sync waits
                k = _touch_n[0] = _touch_n[0] + 1
                nc.vector.tensor_copy(tscr[:][32:33, k % 64:k % 64 + 1], ap)

            # ---------------- input DMAs ----------------
            # f2s first (feature chain blocks on all of it)
            for p0 in range(0, 128, 32):
                nc.sync.dma_start(raw2[:][p0:p0 + 32], f2s_d.ap()[p0:p0 + 32])
            for p0 in range(0, 128, 32):
                nc.sync.dma_start(raw1[:][p0:p0 + 32], f1s_d.ap()[p0:p0 + 32])
            # consts: early piece (masks+bands) separate from the argmax
            # weight table so the big table stays off the critical path
            NC0 = XL + GXL + 512
            nc.sync.dma_start(consts[:][:, 0:NC0], consts_d.ap()[:, 0:NC0])
            nc.sync.dma_start(consts[:][:, NC0:], consts_d.ap()[:, NC0:])
            touch(consts[:][32:33, 0:1])
            xmask = consts[:][:, 0:XL].rearrange("p (q u) -> p q u", q=4)
            gmask = consts[:][:, XL:XL + GXL]
            bands = consts[:][:, XL + GXL:XL + GXL + 512].rearrange(
                "p (t y) -> p t y", t=4)
            wtab = consts[:][:, XL + GXL + 512:].rearrange(
                "p (c s g) -> p c s g", c=2, s=16)

            # ---------------- frame2 features + normalize ----------------
            _emit_features_ph(nc, raw2, sd2, 4)
            s2v = sd2[:][:, :, 0, :, :]
            d2v = sd2[:][:, :, 1, :, :]
            # cross-partition row shifts for the vertical sobel pass
            nc.vector.memset(sdm1[:][0:1], 0.0)
            nc.gpsimd.dma_start(sdm1[:][1:64], sd2[:][0:63, 3, :, :, :])
            nc.gpsimd.dma_start(sdm1[:][64:128], sd2[:][63:127, 3, :, :, :])
            nc.vector.memset(sdp1[:][96:128], 0.0)
            nc.gpsimd.dma_start(sdp1[:][0:64], sd2[:][1:65, 0, :, :, :])
            nc.gpsimd.dma_start(sdp1[:][64:127], sd2[:][65:128, 0, :, :, :])
            fxp2 = raw2[:][:, :, 1, :, :]
            fyp2 = raw2[:][:, :, 2, :, :]

            def vconv(fxp, fyp, dv, sv, ry, dm1, dp1, sm1, sp1):
                nc.vector.scalar_tensor_tensor(fxp[:, ry], dv[:, ry], 2.0,
                                               dm1, Alu.mult, Alu.add)
                nc.vector.tensor_tensor(fxp[:, ry], fxp[:, ry], dp1, Alu.add)
                nc.vector.tensor_tensor(fyp[:, ry], sp1, sm1, Alu.subtract)

            # ry=1,2 batched (neighbors live inside sd2)
            nc.vector.scalar_tensor_tensor(fxp2[:, 1:3], d2v[:, 1:3], 2.0,
                                           d2v[:, 0:2], Alu.mult, Alu.add)
            nc.vector.tensor_tensor(fxp2[:, 1:3], fxp2[:, 1:3], d2v[:, 2:4],
                                    Alu.add)
            nc.vector.tensor_tensor(fyp2[:, 1:3], s2v[:, 2:4], s2v[:, 0:2],
                                    Alu.subtract)
            vconv(fxp2, fyp2, d2v, s2v, 3, d2v[:, 2], sdp1[:][:, 1],
                  s2v[:, 2], sdp1[:][:, 0])
            vconv(fxp2, fyp2, d2v, s2v, 0, sdm1[:][:, 1], d2v[:, 1],
                  sdm1[:][:, 0], s2v[:, 1])

            # normalize: q = g^2+fx^2+fy^2 (squares on ACT), rsqrt + Newton
            nc.scalar.activation(q_t[:], raw2[:][:, :, 0, :, :], Act.Square)
            nc.scalar.activation(r0_t[:], raw2[:][:, :, 1, :, :], Act.Square)
            nc.scalar.activation(a_t[:], raw2[:][:, :, 2, :, :], Act.Square)
            nc.vector.tensor_tensor(q_t[:], q_t[:], r0_t[:], Alu.add)
            # q = (fy^2 max 1e-24) + (g^2 + fx^2): same zero-pixel guard as
            # max(q, 1e-24) since all terms are >= 0
            nc.vector.scalar_tensor_tensor(q_t[:], a_t[:], 1e-24, q_t[:],
                                           Alu.max, Alu.add)
            nc.scalar.activation(r0_t[:], q_t[:], Act.Abs_reciprocal_sqrt)
            nc.vector.tensor_tensor(a_t[:], r0_t[:], r0_t[:], Alu.mult)
            nc.vector.tensor_tensor(a_t[:], a_t[:], q_t[:], Alu.mult)
            nc.vector.tensor_scalar(a_t[:], a_t[:], -0.5, 1.5, Alu.mult,
                                    Alu.add)
            nc.vector.tensor_tensor(r0_t[:], r0_t[:], a_t[:], Alu.mult)
            # ---------------- f2px replication ----------------
            # out-of-image rows: gray=NEG, fx/fy=0 (overwritten where valid);
            # corner masks on the Pool engine (DVE stays on the main chain)
            f2flat = f2px[:].rearrange("p s c q u -> p s (c q u)")
            for (pa, pb, sa, sb) in ((0, 2, 1, 4), (0, 1, 4, 8),
                                     (96, 128, 12, 16)):
                nc.gpsimd.memset(f2flat[pa:pb, sa:sb, 0:XL], float(NEG))
                nc.gpsimd.memset(f2flat[pa:pb, sa:sb, XL:3 * XL], 0.0)
            # fx/fy normalized first so their (bigger) replication share
            # hits the DMA queues before the gray plane is even ready
            for c in (1, 2):
                nc.vector.tensor_tensor(raw2[:][:, :, c, :, :],
                                        raw2[:][:, :, c, :, :],
                                        r0_t[:], Alu.mult)
            REP = ((0, 1, 4, 1), (1, 4, 8, 0), (3, 12, 16, 0))

            def rep_pieces(csel):
                # slot s holds rows 4(v+ov)+ry, s = 4*ovi+ry, ov = ovi-2
                for (ovi, sa, sb, ra) in REP:
                    ov = ovi - 2
                    p0, p1 = max(0, -ov), min(128, 128 - ov)
                    for q0 in range(0, 128, 32):
                        a, b = max(p0, q0), min(p1, q0 + 32)
                        if a < b:
                            nc.sync.dma_start(
                                f2px[:][a:b, sa:sb, csel, :, :],
                                raw2[:][a + ov:b + ov, ra:4, csel, :, :])

            rep_pieces(slice(1, 3))
            nc.vector.tensor_tensor(raw2[:][:, :, 0, :, :],
                                    raw2[:][:, :, 0, :, :],
                                    r0_t[:], Alu.mult)
            # column-validity mask on the gray plane
            nc.vector.tensor_tensor(
                raw2[:][:, :, 0, :, :], raw2[:][:, :, 0, :, :],
                xmask.unsqueeze(1).broadcast_to([128, 4, 4, U72]), Alu.min)
            rep_pieces(slice(0, 1))

            # ---------------- frame1 features (anchor rows only) ----------
            _emit_features_ph(nc, raw1, sd1, 3)
            s1v = sd1[:][:, :, 0, :, :]
            d1v = sd1[:][:, :, 1, :, :]
            nc.vector.memset(sdm1f1[:][0:1], 0.0)
            nc.gpsimd.dma_start(sdm1f1[:][1:64], sd1[:][0:63, 2, :, :, :])
            nc.gpsimd.dma_start(sdm1f1[:][64:128], sd1[:][63:127, 2, :, :, :])
            fxp1 = raw1[:][:, :, 1, :, :]
            fyp1 = raw1[:][:, :, 2, :, :]
            # ry planes stored (0,1,3): vconv for ry=0 uses planes 0,1 and
            # the (v-1, ry=3) shift
            vconv(fxp1, fyp1, d1v, s1v, 0, sdm1f1[:][:, 1], d1v[:, 1],
                  sdm1f1[:][:, 0], s1v[:, 1])

            # ---------------- correlation ----------------
            nc.gpsimd.memset(mx[:], float(NEG))

            def f1bc(c, nu, ns):
                return raw1[:][:, 0, c, 0, 2:70].unsqueeze(1).unsqueeze(1) \
                    .broadcast_to([128, nu, ns, GXL])

            def qparams(q):
                return (1, 3) if q == 0 else (0, 4)

            def products(out, src5, s_lo, s_hi, q, ubase, nu):
                ns = s_hi - s_lo
                nc.vector.tensor_tensor(
                    out, f1bc(0, nu, ns), _win(src5, s_lo, s_hi, 0, q,
                                               ubase, nu), Alu.mult)
                pr = prod[:][:, 0:nu, 0:ns, :]
                nc.vector.tensor_tensor(
                    pr, f1bc(1, nu, ns), _win(src5, s_lo, s_hi, 1, q,
                                              ubase, nu), Alu.mult)
                nc.vector.tensor_tensor(out, out, pr, Alu.add)
                nc.vector.tensor_tensor(
                    pr, f1bc(2, nu, ns), _win(src5, s_lo, s_hi, 2, q,
                                              ubase, nu), Alu.mult)
                nc.vector.tensor_tensor(out, out, pr, Alu.add)

            colmax = mx[:][:, 0, :, :]          # [128, 16, GXL], p = 4q+u0
            rowmax = mx[:][:, 1, :, :]          # [128, 16, GXL], slot s

            # early phase: slots 8..11 (ov=0) read feat2 directly,
            # overlapping the f2px replication DMAs
            for q in range(4):
                ubase, nu = qparams(q)
                cE = corrE[:][:, 0:nu, :, :]
                products(cE, raw2[:], 0, 4, q, ubase, nu)
                for u0 in range(nu):
                    nc.vector.tensor_tensor(rowmax[:, 8:12, :],
                                            rowmax[:, 8:12, :],
                                            cE[:, u0, :, :], Alu.max)
                t2 = tA[:][:, 0:nu, 0:2, :]
                nc.vector.tensor_tensor(t2, cE[:, :, 0:2, :],
                                        cE[:, :, 2:4, :], Alu.max)
                nc.vector.tensor_tensor(colmax[:, 4 * q + ubase:4 * q + 4, :],
                                        t2[:, :, 0, :], t2[:, :, 1, :],
                                        Alu.max)

            # late phase: slots 1..7 and 12..15 via f2px
            for q in range(4):
                ubase, nu = qparams(q)
                c7 = corrL[:][:, 0:nu, 1:8, :]
                c4 = corrL[:][:, 0:nu, 12:16, :]
                products(c7, f2px[:], 1, 8, q, ubase, nu)
                products(c4, f2px[:], 12, 16, q, ubase, nu)
                for u0 in range(nu):
                    nc.vector.tensor_tensor(rowmax[:, 1:8, :],
                                            rowmax[:, 1:8, :],
                                            c7[:, u0, :, :], Alu.max)
                    nc.vector.tensor_tensor(rowmax[:, 12:16, :],
                                            rowmax[:, 12:16, :],
                                            c4[:, u0, :, :], Alu.max)
                # colmax tree over slots 1..7
                a3 = tA[:][:, 0:nu, :, :]
                nc.vector.tensor_tensor(a3, c7[:, :, 0:3, :],
                                        c7[:, :, 3:6, :], Alu.max)
                b1 = tB[:][:, 0:nu, :]
                nc.vector.tensor_tensor(b1, a3[:, :, 0, :], a3[:, :, 1, :],
                                        Alu.max)
                nc.vector.tensor_tensor(b1, b1, a3[:, :, 2, :], Alu.max)
                nc.vector.tensor_tensor(b1, b1, c7[:, :, 6, :], Alu.max)
                cm = colmax[:, 4 * q + ubase:4 * q + 4, :]
                nc.vector.tensor_tensor(cm, cm, b1, Alu.max)
                # colmax tree over slots 12..15
                t2 = tA[:][:, 0:nu, 0:2, :]
                nc.vector.tensor_tensor(t2, c4[:, :, 0:2, :],
                                        c4[:, :, 2:4, :], Alu.max)
                nc.vector.tensor_tensor(b1, t2[:, :, 0, :], t2[:, :, 1, :],
                                        Alu.max)
                nc.vector.tensor_tensor(cm, cm, b1, Alu.max)

            # ---------------- batched first-argmax -> grid ----------------
            t8 = tr[:][:, :, 0:8, :]
            nc.vector.tensor_tensor(t8, mx[:][:, :, 0:8, :],
                                    mx[:][:, :, 8:16, :], Alu.max)
            t4 = tr[:][:, :, 0:4, :]
            nc.vector.tensor_tensor(t4, t8[:, :, 0:4, :], t8[:, :, 4:8, :],
                                    Alu.max)
            nc.vector.tensor_tensor(t4[:, :, 0:2, :], t4[:, :, 0:2, :],
                                    t4[:, :, 2:4, :], Alu.max)
            nc.vector.tensor_tensor(m2[:], t4[:, :, 0, :], t4[:, :, 1, :],
                                    Alu.max)
            mb = m2[:].unsqueeze(2).broadcast_to([128, 2, 16, GXL])
            nc.vector.tensor_tensor(wsum[:], mx[:], mb, Alu.is_ge)
            nc.vector.tensor_tensor(wsum[:], wsum[:], wtab, Alu.mult)
            nc.vector.tensor_tensor(t8, wsum[:][:, :, 0:8, :],
                                    wsum[:][:, :, 8:16, :], Alu.max)
            nc.vector.tensor_tensor(t4, t8[:, :, 0:4, :], t8[:, :, 4:8, :],
                                    Alu.max)
            nc.vector.tensor_tensor(t4[:, :, 0:2, :], t4[:, :, 0:2, :],
                                    t4[:, :, 2:4, :], Alu.max)
            nc.vector.tensor_tensor(fm2[:], t4[:, :, 0, :], t4[:, :, 1, :],
                                    Alu.max)
            # disp = (8 - fm)/512 ; zero invalid anchors
            nc.vector.tensor_scalar(fm2[:], fm2[:], -1.0 / 512.0, 8.0 / 512.0,
                                    Alu.mult, Alu.add)
            nc.vector.tensor_tensor(
                grid[:], fm2[:],
                gmask.unsqueeze(1).broadcast_to([128, 2, GXL]), Alu.mult)

            # ---------------- smoothing H-pass (phase weights) -------------
            Wp = _phase_weights()
            hsc2 = hsc[:]
            for p in range(4):
                nc.vector.tensor_scalar_mul(
                    hsc2, grid[:][:, :, 0:64], float(Wp[p, 0]))
                for dd in range(1, 4):
                    nc.vector.scalar_tensor_tensor(
                        hsc2, grid[:][:, :, dd:dd + 64],
                        float(Wp[p, dd]), hsc2, Alu.mult, Alu.add)
                nc.vector.scalar_tensor_tensor(
                    hp[:][:, :, p:256:4], grid[:][:, :, 4:4 + 64],
                    float(Wp[p, 4]), hsc2, Alu.mult, Alu.add)

            # ---------------- V-pass (PE banded matmul) + normalize --------
            nc.vector.tensor_copy(bands2[:], bands)
            nc.scalar.copy(tscr[:][32:33, 0:1], hp[:][32:33, 0, 0:1])
            rhs = hp[:].rearrange("p c x -> p (c x)")
            for t in range(4):
                ps = psum_pool.tile([128, 512], F32, tag="vps")
                nc.tensor.matmul(ps[:], bands2[:][:, t, :], rhs,
                                 start=True, stop=True)
                sm = smsb[t]
                nc.scalar.copy(sm[:].rearrange("p c x -> p (c x)"), ps[:])
                psv = sm[:]
                nq1, nq2, nr, nm = nq1s[t], nq2s[t], nrs[t], nms[t]
                nc.scalar.activation(nq1[:], psv[:, 0, :], Act.Square)
                nc.scalar.activation(nq2[:], psv[:, 1, :], Act.Square)
                nc.vector.scalar_tensor_tensor(nq1[:], nq1[:], 1e-30, nq2[:],
                                               Alu.max, Alu.add)
                # mag = q*rsqrt(q); d = max(mag,1e-6)+1e-6; out = flow/d
                # (raw ACT rsqrt, ~5e-4 rel err: well under tolerance)
                nc.scalar.activation(nr[:], nq1[:], Act.Abs_reciprocal_sqrt)
                nc.vector.tensor_tensor(nm[:], nq1[:], nr[:], Alu.mult)
                nc.vector.tensor_scalar(nm[:], nm[:], 1e-6, 1e-6, Alu.max,
                                        Alu.add)
                nc.vector.tensor_tensor(nq2[:], nm[:], nm[:], Alu.mult)
                nc.scalar.activation(nr[:], nq2[:], Act.Abs_reciprocal_sqrt)
                nc.vector.tensor_tensor(outsb[:][:, t, 0, :], psv[:, 0, :],
                                        nr[:], Alu.mult)
                nc.vector.tensor_tensor(outsb[:][:, t, 1, :], psv[:, 1, :],
                                        nr[:], Alu.mult)
                # stream this t-plane out while later planes still compute
                for p0 in range(0, 128, 32):
                    nc.sync.dma_start(out_d.ap()[p0:p0 + 32, t],
                                      outsb[:][p0:p0 + 32, t])

    nc.compile()
    return nc


_NC_CACHE = None


def _get_nc():
    global _NC_CACHE
    if _NC_CACHE is None:
        _NC_CACHE = build_program()
    return _NC_CACHE


def kernel(frame1, frame2):
    frame1 = np.asarray(frame1, dtype=np.float32)
    frame2 = np.asarray(frame2, dtype=np.float32)
    nc = _get_nc()
    in_maps = _host_inputs(frame1, frame2)
    res = run_bass_kernel_spmd(nc, in_maps, core_ids=list(range(8)))
    if res.exec_time_ns is not None:
        print(f"HW exec time: {res.exec_time_ns} ns")
    out = np.empty((B, 2, H, W), np.float32)
    for b in range(B):
        for w in range(2):
            o = res.results[2 * b + w]["out"]        # [128, 4, 2, 256]
            o = o.transpose(2, 1, 0, 3).reshape(2, H, 256)
            out[b, :, :, 256 * w:256 * w + 256] = o
    return out
